# revision 14
# baseline (speedup 1.0000x reference)
"""Trainium2 Bass kernel for nn_OneDimEquivalent (sequential scalar recurrence).

Math: for t = 0..T-1 with state (k, v) starting at (0, 0):
    delta2 = SIG_M^2 k^2 + SIG_I^2 u_t^2
    gi     = G(delta2)          (Gauss-Legendre-64 integral in the reference)
    k'     = (1-r) k + r*SIG_MN*gi*k + r*SIG_NI*gi*v
    v'     = (1-r) v + r u_t
    z_t    = SIG_MW * gi * k'
Output: [0, z_0 .. z_{T-1}]  (length T+1).

Device algorithm (per core, data-parallel over 8 cores):
  The recurrence contracts (|dk'/dk| <= ~0.87), so each core independently
  processes its T/8 slice plus a W-step warmup prefix. Within a core the
  window is laid out as 128 rows x C cols; the nonlinear recurrence is solved
  by Picard iteration: freeze gi along the current trajectory, then the k
  update is a linear first-order recurrence solved in one tensor_tensor_scan
  per iteration (row chaining via previous iteration's row tails, shifted
  across partitions with a tiny PE matmul). gi = G(delta2) is evaluated as
  G0*exp(-F(y)), y = ln(1 + 3*delta2), F a degree-7 polynomial fit of the
  reference's own quadrature (rel err 1.5e-5). 5 iterations converge to the
  fit floor. v is exact: two chained scans.
"""

import os
import sys

import numpy as np

for _p in ("/opt/trn_rl_repo",):
    if _p not in sys.path and os.path.isdir(_p):
        sys.path.insert(0, _p)

import concourse.bass as bass  # noqa: E402
from concourse import bacc  # noqa: E402
import concourse.tile as tile  # noqa: E402
from concourse import mybir  # noqa: E402
from concourse.bass_utils import run_bass_kernel_spmd  # noqa: E402

F32 = mybir.dt.float32

T = 524288
NCORES = 8
TC = T // NCORES          # 65536 outputs per core
P = 128                   # partitions
W = 256                   # warmup prefix (contraction 0.87^256 ~ 0)
CW = TC + W               # window elements per core
C = CW // P               # columns per row (514)
N_ITER = 5

R = 0.2                   # DT / TAU
ONE_MR = 1.0 - R
SIG_M = 1.2               # Square scale for 1.44 k^2
SIG_I = 0.8               # Square scale for 0.64 u^2
BETA = 0.5 / 0.9          # SIG_NI / SIG_MN
C6 = 0.7 / (R * 0.9)      # SIG_MW / (r SIG_MN)
ALPHA = 3.0               # y = ln(1 + ALPHA * delta2)

# g1 = r*SIG_MN*G(delta2) = exp(-F~ + EXP_BIAS), F~ = y*Q(y)
EXP_BIAS = -2.633724671491273
# Horner constants, top-first: h = QH[0]*y; h = (h + QH[i])*y ...
QH = (
    3.907613360649551e-05,
    -0.0005464510002932087,
    0.003347948402461663,
    -0.011995295768374504,
    0.024299061938897772,
    0.0028116659750275206,
    0.33319012156717015,
)

_CACHE: dict = {}

VARIANT = int(os.environ.get("KERNEL_VARIANT", "2"))
NW = 2                    # windows per core (v2)
TC2 = TC // NW            # 32768
CW2 = TC2 + W             # 33024
C2 = CW2 // P             # 258
N_ITER2 = 4


def _build_program_v2() -> bass.Bass:
    """Two interleaved half-windows per core: pipelines the serial chain
    across DVE/ACT/Pool/PE; d2-add and b-mult offloaded to gpsimd."""
    from contextlib import ExitStack

    AF = mybir.ActivationFunctionType
    OP = mybir.AluOpType

    nc = bacc.Bacc()
    u_win_d = nc.declare_dram_parameter("u_win", [P, NW * C2], F32, isOutput=False)
    u_sh_d = nc.declare_dram_parameter("u_sh", [P, NW * C2], F32, isOutput=False)
    z_d = nc.declare_dram_parameter("z", [P, NW * C2], F32, isOutput=True)

    with ExitStack() as ctx:
        tc = ctx.enter_context(tile.TileContext(nc))
        pool = ctx.enter_context(tc.tile_pool(name="main", bufs=1))

        def t(tag, cols=C2):
            return pool.tile([P, cols], F32, tag=tag, name=tag)

        WS = range(NW)
        uw = [t(f"uw{w}") for w in WS]
        ush = [t(f"ush{w}") for w in WS]
        U2 = [t(f"U2_{w}") for w in WS]
        bv = [t(f"bv{w}") for w in WS]
        av = t("av")                      # shared constant 0.8 tile
        vsh = [t(f"vsh{w}") for w in WS]
        V2 = [t(f"V2_{w}") for w in WS]
        k = [t(f"k{w}") for w in WS]
        s = [t(f"s{w}") for w in WS]
        d2 = [t(f"d2_{w}") for w in WS]
        y = [t(f"y{w}") for w in WS]
        ha = [t(f"ha{w}") for w in WS]
        hb = [t(f"hb{w}") for w in WS]
        g1 = [t(f"g1_{w}") for w in WS]
        a = [t(f"a{w}") for w in WS]
        b = [t(f"b{w}") for w in WS]
        zt = [t(f"zt{w}") for w in WS]
        ebias = pool.tile([P, 1], F32, tag="ebias", name="ebias")
        # partition-shifted row tails (row 0 stays 0 = cold window start)
        vtail = [
            pool.tile([P, 1], F32, tag=f"vtail{w}", name=f"vtail{w}") for w in WS
        ]
        ktail = [
            pool.tile([P, 1], F32, tag=f"ktail{w}", name=f"ktail{w}") for w in WS
        ]

        for w in WS:
            cs = slice(w * C2, (w + 1) * C2)
            nc.sync.dma_start(uw[w][:], u_win_d[:, cs])
            nc.sync.dma_start(ush[w][:], u_sh_d[:, cs])
        nc.vector.memset(ebias[:], EXP_BIAS)
        nc.vector.memset(av[:], ONE_MR)

        # ---- setup per window ----
        for w in WS:
            nc.scalar.activation(U2[w][:], uw[w][:], AF.Square, scale=SIG_I)
            nc.vector.tensor_scalar_mul(bv[w][:], ush[w][:], R)
            nc.vector.memset(vtail[w][:], 0.0)
            nc.vector.memset(ktail[w][:], 0.0)
        for w in WS:
            nc.vector.tensor_tensor_scan(
                vsh[w][:], av[:], bv[w][:], 0.0, OP.mult, OP.add
            )
            nc.sync.dma_start(
                vtail[w][1:P, 0:1], vsh[w][0 : P - 1, C2 - 1 : C2]
            )
            nc.vector.tensor_tensor_scan(
                vsh[w][:], av[:], bv[w][:], vtail[w][:], OP.mult, OP.add
            )
            nc.vector.tensor_scalar_mul(V2[w][:], vsh[w][:], BETA)
            nc.vector.memset(k[w][:], 0.0)

        # ---- Picard iterations ----
        for it in range(N_ITER2):
            for w in WS:
                if it > 0:
                    nc.sync.dma_start(
                        ktail[w][1:P, 0:1], k[w][0 : P - 1, C2 - 1 : C2]
                    )
            for w in WS:
                nc.scalar.activation(
                    s[w][:, 0:1], ktail[w][:], AF.Square, scale=SIG_M
                )
                nc.scalar.activation(
                    s[w][:, 1:C2], k[w][:, 0 : C2 - 1], AF.Square, scale=SIG_M
                )
            for w in WS:
                nc.gpsimd.tensor_tensor(d2[w][:], s[w][:], U2[w][:], OP.add)
            for w in WS:
                nc.scalar.activation(y[w][:], d2[w][:], AF.Ln, scale=ALPHA, bias=1.0)
            for w in WS:
                nc.vector.tensor_scalar_mul(ha[w][:], y[w][:], QH[0])
            srcs, dsts = list(ha), list(hb)
            for q in QH[1:]:
                for w in WS:
                    nc.vector.scalar_tensor_tensor(
                        dsts[w][:], srcs[w][:], float(q), y[w][:], OP.add, OP.mult
                    )
                srcs, dsts = dsts, srcs
            for w in WS:
                nc.scalar.activation(
                    g1[w][:], srcs[w][:], AF.Exp, scale=-1.0, bias=ebias[:]
                )
            for w in WS:
                nc.vector.tensor_scalar_add(a[w][:], g1[w][:], ONE_MR)
                nc.gpsimd.tensor_tensor(b[w][:], g1[w][:], V2[w][:], OP.mult)
            for w in WS:
                nc.vector.tensor_tensor_scan(
                    k[w][:], a[w][:], b[w][:], ktail[w][:], OP.mult, OP.add
                )

        # ---- output ----
        for w in WS:
            nc.vector.scalar_tensor_tensor(
                zt[w][:], g1[w][:], C6, k[w][:], OP.mult, OP.mult
            )
            nc.sync.dma_start(z_d[:, w * C2 : (w + 1) * C2], zt[w][:])
    return nc


def _build_program() -> bass.Bass:
    from contextlib import ExitStack

    AF = mybir.ActivationFunctionType
    OP = mybir.AluOpType

    nc = bacc.Bacc()
    u_win_d = nc.declare_dram_parameter("u_win", [P, C], F32, isOutput=False)
    u_sh_d = nc.declare_dram_parameter("u_sh", [P, C], F32, isOutput=False)
    sm_d = nc.declare_dram_parameter("shiftm", [P, P], F32, isOutput=False)
    z_d = nc.declare_dram_parameter("z", [P, C], F32, isOutput=True)

    with ExitStack() as ctx:
        tc = ctx.enter_context(tile.TileContext(nc))
        pool = ctx.enter_context(tc.tile_pool(name="main", bufs=1))
        psum = ctx.enter_context(tc.tile_pool(name="ps", bufs=2, space="PSUM"))

        def big(tag):
            return pool.tile([P, C], F32, tag=tag, name=tag)

        uw = big("uw")
        ush = big("ush")
        U2 = big("U2")
        bv = big("bv")
        av = big("av")
        vsh = big("vsh")
        V2 = big("V2")
        k = big("k")
        s = big("s")
        d2 = big("d2")
        y = big("y")
        ha = big("ha")
        hb = big("hb")
        g1 = big("g1")
        a = big("a")
        b = big("b")
        zt = big("zt")
        sm = pool.tile([P, P], F32, tag="sm", name="sm")
        ebias = pool.tile([P, 1], F32, tag="ebias", name="ebias")

        nc.sync.dma_start(uw[:], u_win_d[:])
        nc.sync.dma_start(ush[:], u_sh_d[:])
        nc.sync.dma_start(sm[:], sm_d[:])

        # ---- setup: U2 = 0.64 u^2 ; vsh = v_{t-1}; V2 = BETA*vsh ----
        nc.vector.memset(ebias[:], EXP_BIAS)
        nc.scalar.activation(U2[:], uw[:], AF.Square, scale=SIG_I)
        nc.vector.tensor_scalar_mul(bv[:], ush[:], R)
        nc.vector.memset(av[:], ONE_MR)
        nc.vector.tensor_tensor_scan(vsh[:], av[:], bv[:], 0.0, OP.mult, OP.add)
        pv = psum.tile([P, 1], F32, tag="pv", name="pv")
        nc.tensor.matmul(pv[:], sm[:], vsh[:, C - 1 : C], start=True, stop=True)
        nc.vector.tensor_tensor_scan(vsh[:], av[:], bv[:], pv[:], OP.mult, OP.add)
        nc.vector.tensor_scalar_mul(V2[:], vsh[:], BETA)
        nc.vector.memset(k[:], 0.0)

        # ---- Picard iterations ----
        for _it in range(N_ITER):
            pt = psum.tile([P, 1], F32, tag="pt", name="pt")
            nc.tensor.matmul(pt[:], sm[:], k[:, C - 1 : C], start=True, stop=True)
            nc.scalar.activation(s[:, 0:1], pt[:], AF.Square, scale=SIG_M)
            nc.scalar.activation(s[:, 1:C], k[:, 0 : C - 1], AF.Square, scale=SIG_M)
            nc.vector.tensor_tensor(d2[:], s[:], U2[:], OP.add)
            nc.scalar.activation(y[:], d2[:], AF.Ln, scale=ALPHA, bias=1.0)
            nc.vector.tensor_scalar_mul(ha[:], y[:], QH[0])
            src, dst = ha, hb
            for q in QH[1:]:
                nc.vector.scalar_tensor_tensor(
                    dst[:], src[:], float(q), y[:], OP.add, OP.mult
                )
                src, dst = dst, src
            nc.scalar.activation(g1[:], src[:], AF.Exp, scale=-1.0, bias=ebias[:])
            nc.vector.tensor_scalar_add(a[:], g1[:], ONE_MR)
            nc.vector.tensor_tensor(b[:], g1[:], V2[:], OP.mult)
            nc.vector.tensor_tensor_scan(k[:], a[:], b[:], pt[:], OP.mult, OP.add)

        # ---- output: z = (g1 * C6) * k ----
        nc.vector.scalar_tensor_tensor(zt[:], g1[:], C6, k[:], OP.mult, OP.mult)
        nc.sync.dma_start(z_d[:], zt[:])
    return nc


def _get_nc() -> bass.Bass:
    if "nc" not in _CACHE:
        nc = _build_program_v2() if VARIANT == 2 else _build_program()
        # Bacc lowering (register allocation, wait legalization) must run
        # before the PJRT path serializes the module.
        nc.finalize()
        _CACHE["nc"] = nc
    return _CACHE["nc"]


def _make_in_maps(u: np.ndarray) -> list[dict]:
    u_pad = np.zeros(T + W, np.float32)
    u_pad[W:] = u
    ush_pad = np.zeros(T + W, np.float32)
    ush_pad[1:] = u_pad[:-1]
    shiftm = np.eye(P, k=1, dtype=np.float32)
    in_maps = []
    if VARIANT == 2:
        # window j of core c covers [c*TC + j*TC2 - W, c*TC + (j+1)*TC2)
        u_pad2 = np.zeros(T + W, np.float32)
        u_pad2[W:] = u
        for c in range(NCORES):
            uws, ushs = [], []
            for j in range(NW):
                lo = c * TC + j * TC2
                uws.append(u_pad2[lo : lo + CW2].reshape(P, C2))
                ushs.append(ush_pad[lo : lo + CW2].reshape(P, C2))
            in_maps.append(
                {
                    "u_win": np.ascontiguousarray(np.concatenate(uws, axis=1)),
                    "u_sh": np.ascontiguousarray(np.concatenate(ushs, axis=1)),
                }
            )
        return in_maps
    for c in range(NCORES):
        lo = c * TC
        in_maps.append(
            {
                "u_win": np.ascontiguousarray(u_pad[lo : lo + CW].reshape(P, C)),
                "u_sh": np.ascontiguousarray(ush_pad[lo : lo + CW].reshape(P, C)),
                "shiftm": shiftm,
            }
        )
    return in_maps


def _assemble(results: list[dict]) -> np.ndarray:
    z = np.zeros(T + 1, np.float32)
    if VARIANT == 2:
        for c in range(NCORES):
            zc = results[c]["z"]
            for j in range(NW):
                lo = c * TC + j * TC2
                zj = zc[:, j * C2 : (j + 1) * C2].reshape(-1)[W:]
                z[lo + 1 : lo + TC2 + 1] = zj
        return z
    for c in range(NCORES):
        z[c * TC + 1 : (c + 1) * TC + 1] = results[c]["z"].reshape(-1)[W:]
    return z


def kernel(u: np.ndarray, _trace: bool = False):
    u = np.asarray(u, dtype=np.float32).reshape(-1)
    assert u.shape[0] == T, u.shape
    in_maps = _make_in_maps(u)
    res = run_bass_kernel_spmd(
        _get_nc(), in_maps, list(range(NCORES)), trace=_trace
    )
    _CACHE["last_result"] = res
    return _assemble(res.results)


# revision 16
# speedup vs baseline: 1.0568x; 1.0568x over previous
"""Trainium2 Bass kernel for nn_OneDimEquivalent (sequential scalar recurrence).

Math: for t = 0..T-1 with state (k, v) starting at (0, 0):
    delta2 = SIG_M^2 k^2 + SIG_I^2 u_t^2
    gi     = G(delta2)          (Gauss-Legendre-64 integral in the reference)
    k'     = (1-r) k + r*SIG_MN*gi*k + r*SIG_NI*gi*v
    v'     = (1-r) v + r u_t
    z_t    = SIG_MW * gi * k'
Output: [0, z_0 .. z_{T-1}]  (length T+1).

Device algorithm (per core, data-parallel over 8 cores):
  The recurrence contracts (|dk'/dk| <= ~0.87), so each core independently
  processes its T/8 slice plus a W-step warmup prefix. Within a core the
  window is laid out as 128 rows x C cols; the nonlinear recurrence is solved
  by Picard iteration: freeze gi along the current trajectory, then the k
  update is a linear first-order recurrence solved in one tensor_tensor_scan
  per iteration (row chaining via previous iteration's row tails, shifted
  across partitions with a tiny PE matmul). gi = G(delta2) is evaluated as
  G0*exp(-F(y)), y = ln(1 + 3*delta2), F a degree-7 polynomial fit of the
  reference's own quadrature (rel err 1.5e-5). 5 iterations converge to the
  fit floor. v is exact: two chained scans.
"""

import os
import sys

import numpy as np

for _p in ("/opt/trn_rl_repo",):
    if _p not in sys.path and os.path.isdir(_p):
        sys.path.insert(0, _p)

import concourse.bass as bass  # noqa: E402
from concourse import bacc  # noqa: E402


def _pin_act_tables() -> None:
    """All three ACT functions used here (Square, Ln, Exp) live together in
    the natural_log_exp_and_others set, but the table-load inserter picks the
    first set containing each function, which alternates sets and reloads the
    ACT table RAMs (~1.3us) twice per iteration. Strip our functions from
    every other set (ids keep their positions) so one load serves the whole
    kernel."""
    if getattr(bacc, "_act_tables_pinned", False):
        return
    from concourse.hw_specs import get_activation_tables as _orig

    AF = mybir.ActivationFunctionType
    mine = {AF.Square, AF.Ln, AF.Exp}

    def pinned(arch):
        tabs = _orig(arch)
        out = {}
        for name, fns in tabs.items():
            if name == "natural_log_exp_and_others":
                out[name] = fns
            else:
                out[name] = fns - mine
        return out

    bacc.get_activation_tables = pinned
    bacc._act_tables_pinned = True
import concourse.tile as tile  # noqa: E402
from concourse import mybir  # noqa: E402
from concourse.bass_utils import run_bass_kernel_spmd  # noqa: E402

F32 = mybir.dt.float32

T = 524288
NCORES = 8
TC = T // NCORES          # 65536 outputs per core
P = 128                   # partitions
W = 256                   # warmup prefix (contraction 0.87^256 ~ 0)
CW = TC + W               # window elements per core
C = CW // P               # columns per row (514)
N_ITER = 5

R = 0.2                   # DT / TAU
ONE_MR = 1.0 - R
SIG_M = 1.2               # Square scale for 1.44 k^2
SIG_I = 0.8               # Square scale for 0.64 u^2
BETA = 0.5 / 0.9          # SIG_NI / SIG_MN
C6 = 0.7 / (R * 0.9)      # SIG_MW / (r SIG_MN)
ALPHA = 3.0               # y = ln(1 + ALPHA * delta2)

# g1 = r*SIG_MN*G(delta2) = exp(-F~ + EXP_BIAS), F~ = y*Q(y)
EXP_BIAS = -2.633724671491273
# Horner constants, top-first: h = QH[0]*y; h = (h + QH[i])*y ...
QH = (
    3.907613360649551e-05,
    -0.0005464510002932087,
    0.003347948402461663,
    -0.011995295768374504,
    0.024299061938897772,
    0.0028116659750275206,
    0.33319012156717015,
)

_CACHE: dict = {}

VARIANT = int(os.environ.get("KERNEL_VARIANT", "2"))
NW = 2                    # windows per core (v2)
TC2 = TC // NW            # 32768
CW2 = TC2 + W             # 33024
C2 = CW2 // P             # 258
N_ITER2 = 4


def _build_program_v2() -> bass.Bass:
    """Two interleaved half-windows per core: pipelines the serial chain
    across DVE/ACT/Pool/PE; d2-add and b-mult offloaded to gpsimd."""
    from contextlib import ExitStack

    AF = mybir.ActivationFunctionType
    OP = mybir.AluOpType

    nc = bacc.Bacc()
    u_win_d = nc.declare_dram_parameter("u_win", [P, NW * C2], F32, isOutput=False)
    u_sh_d = nc.declare_dram_parameter("u_sh", [P, NW * C2], F32, isOutput=False)
    z_d = nc.declare_dram_parameter("z", [P, NW * C2], F32, isOutput=True)

    with ExitStack() as ctx:
        tc = ctx.enter_context(tile.TileContext(nc))
        pool = ctx.enter_context(tc.tile_pool(name="main", bufs=1))

        def t(tag, cols=C2):
            return pool.tile([P, cols], F32, tag=tag, name=tag)

        WS = range(NW)
        uw = [t(f"uw{w}") for w in WS]
        ush = [t(f"ush{w}") for w in WS]
        U2 = [t(f"U2_{w}") for w in WS]
        bv = [t(f"bv{w}") for w in WS]
        av = t("av")                      # shared constant 0.8 tile
        vsh = [t(f"vsh{w}") for w in WS]
        V2 = [t(f"V2_{w}") for w in WS]
        k = [t(f"k{w}") for w in WS]
        s = [t(f"s{w}") for w in WS]
        d2 = [t(f"d2_{w}") for w in WS]
        y = [t(f"y{w}") for w in WS]
        ha = [t(f"ha{w}") for w in WS]
        hb = [t(f"hb{w}") for w in WS]
        g1 = [t(f"g1_{w}") for w in WS]
        a = [t(f"a{w}") for w in WS]
        b = [t(f"b{w}") for w in WS]
        zt = [t(f"zt{w}") for w in WS]
        ebias = pool.tile([P, 1], F32, tag="ebias", name="ebias")
        # partition-shifted row tails (row 0 stays 0 = cold window start)
        vtail = [
            pool.tile([P, 1], F32, tag=f"vtail{w}", name=f"vtail{w}") for w in WS
        ]
        ktail = [
            pool.tile([P, 1], F32, tag=f"ktail{w}", name=f"ktail{w}") for w in WS
        ]

        for w in WS:
            cs = slice(w * C2, (w + 1) * C2)
            nc.sync.dma_start(uw[w][:], u_win_d[:, cs])
            nc.sync.dma_start(ush[w][:], u_sh_d[:, cs])
        nc.vector.memset(ebias[:], EXP_BIAS)
        nc.vector.memset(av[:], ONE_MR)

        # ---- setup per window ----
        for w in WS:
            nc.scalar.activation(U2[w][:], uw[w][:], AF.Square, scale=SIG_I)
            nc.vector.tensor_scalar_mul(bv[w][:], ush[w][:], R)
            nc.vector.memset(vtail[w][:], 0.0)
            nc.vector.memset(ktail[w][:], 0.0)
        for w in WS:
            nc.vector.tensor_tensor_scan(
                vsh[w][:], av[:], bv[w][:], 0.0, OP.mult, OP.add
            )
            nc.sync.dma_start(
                vtail[w][1:P, 0:1], vsh[w][0 : P - 1, C2 - 1 : C2]
            )
            nc.vector.tensor_tensor_scan(
                vsh[w][:], av[:], bv[w][:], vtail[w][:], OP.mult, OP.add
            )
            nc.vector.tensor_scalar_mul(V2[w][:], vsh[w][:], BETA)
            nc.vector.memset(k[w][:], 0.0)

        # ---- Picard iterations ----
        for it in range(N_ITER2):
            for w in WS:
                if it > 0:
                    nc.sync.dma_start(
                        ktail[w][1:P, 0:1], k[w][0 : P - 1, C2 - 1 : C2]
                    )
            for w in WS:
                nc.scalar.activation(
                    s[w][:, 0:1], ktail[w][:], AF.Square, scale=SIG_M
                )
                nc.scalar.activation(
                    s[w][:, 1:C2], k[w][:, 0 : C2 - 1], AF.Square, scale=SIG_M
                )
            for w in WS:
                nc.gpsimd.tensor_tensor(d2[w][:], s[w][:], U2[w][:], OP.add)
            for w in WS:
                nc.scalar.activation(y[w][:], d2[w][:], AF.Ln, scale=ALPHA, bias=1.0)
            for w in WS:
                nc.vector.tensor_scalar_mul(ha[w][:], y[w][:], QH[0])
            srcs, dsts = list(ha), list(hb)
            for q in QH[1:]:
                for w in WS:
                    nc.vector.scalar_tensor_tensor(
                        dsts[w][:], srcs[w][:], float(q), y[w][:], OP.add, OP.mult
                    )
                srcs, dsts = dsts, srcs
            for w in WS:
                nc.scalar.activation(
                    g1[w][:], srcs[w][:], AF.Exp, scale=-1.0, bias=ebias[:]
                )
            for w in WS:
                nc.vector.tensor_scalar_add(a[w][:], g1[w][:], ONE_MR)
                nc.gpsimd.tensor_tensor(b[w][:], g1[w][:], V2[w][:], OP.mult)
            for w in WS:
                nc.vector.tensor_tensor_scan(
                    k[w][:], a[w][:], b[w][:], ktail[w][:], OP.mult, OP.add
                )

        # ---- output ----
        for w in WS:
            nc.vector.scalar_tensor_tensor(
                zt[w][:], g1[w][:], C6, k[w][:], OP.mult, OP.mult
            )
            nc.sync.dma_start(z_d[:, w * C2 : (w + 1) * C2], zt[w][:])
    return nc


def _build_program() -> bass.Bass:
    from contextlib import ExitStack

    AF = mybir.ActivationFunctionType
    OP = mybir.AluOpType

    nc = bacc.Bacc()
    u_win_d = nc.declare_dram_parameter("u_win", [P, C], F32, isOutput=False)
    u_sh_d = nc.declare_dram_parameter("u_sh", [P, C], F32, isOutput=False)
    sm_d = nc.declare_dram_parameter("shiftm", [P, P], F32, isOutput=False)
    z_d = nc.declare_dram_parameter("z", [P, C], F32, isOutput=True)

    with ExitStack() as ctx:
        tc = ctx.enter_context(tile.TileContext(nc))
        pool = ctx.enter_context(tc.tile_pool(name="main", bufs=1))
        psum = ctx.enter_context(tc.tile_pool(name="ps", bufs=2, space="PSUM"))

        def big(tag):
            return pool.tile([P, C], F32, tag=tag, name=tag)

        uw = big("uw")
        ush = big("ush")
        U2 = big("U2")
        bv = big("bv")
        av = big("av")
        vsh = big("vsh")
        V2 = big("V2")
        k = big("k")
        s = big("s")
        d2 = big("d2")
        y = big("y")
        ha = big("ha")
        hb = big("hb")
        g1 = big("g1")
        a = big("a")
        b = big("b")
        zt = big("zt")
        sm = pool.tile([P, P], F32, tag="sm", name="sm")
        ebias = pool.tile([P, 1], F32, tag="ebias", name="ebias")

        nc.sync.dma_start(uw[:], u_win_d[:])
        nc.sync.dma_start(ush[:], u_sh_d[:])
        nc.sync.dma_start(sm[:], sm_d[:])

        # ---- setup: U2 = 0.64 u^2 ; vsh = v_{t-1}; V2 = BETA*vsh ----
        nc.vector.memset(ebias[:], EXP_BIAS)
        nc.scalar.activation(U2[:], uw[:], AF.Square, scale=SIG_I)
        nc.vector.tensor_scalar_mul(bv[:], ush[:], R)
        nc.vector.memset(av[:], ONE_MR)
        nc.vector.tensor_tensor_scan(vsh[:], av[:], bv[:], 0.0, OP.mult, OP.add)
        pv = psum.tile([P, 1], F32, tag="pv", name="pv")
        nc.tensor.matmul(pv[:], sm[:], vsh[:, C - 1 : C], start=True, stop=True)
        nc.vector.tensor_tensor_scan(vsh[:], av[:], bv[:], pv[:], OP.mult, OP.add)
        nc.vector.tensor_scalar_mul(V2[:], vsh[:], BETA)
        nc.vector.memset(k[:], 0.0)

        # ---- Picard iterations ----
        for _it in range(N_ITER):
            pt = psum.tile([P, 1], F32, tag="pt", name="pt")
            nc.tensor.matmul(pt[:], sm[:], k[:, C - 1 : C], start=True, stop=True)
            nc.scalar.activation(s[:, 0:1], pt[:], AF.Square, scale=SIG_M)
            nc.scalar.activation(s[:, 1:C], k[:, 0 : C - 1], AF.Square, scale=SIG_M)
            nc.vector.tensor_tensor(d2[:], s[:], U2[:], OP.add)
            nc.scalar.activation(y[:], d2[:], AF.Ln, scale=ALPHA, bias=1.0)
            nc.vector.tensor_scalar_mul(ha[:], y[:], QH[0])
            src, dst = ha, hb
            for q in QH[1:]:
                nc.vector.scalar_tensor_tensor(
                    dst[:], src[:], float(q), y[:], OP.add, OP.mult
                )
                src, dst = dst, src
            nc.scalar.activation(g1[:], src[:], AF.Exp, scale=-1.0, bias=ebias[:])
            nc.vector.tensor_scalar_add(a[:], g1[:], ONE_MR)
            nc.vector.tensor_tensor(b[:], g1[:], V2[:], OP.mult)
            nc.vector.tensor_tensor_scan(k[:], a[:], b[:], pt[:], OP.mult, OP.add)

        # ---- output: z = (g1 * C6) * k ----
        nc.vector.scalar_tensor_tensor(zt[:], g1[:], C6, k[:], OP.mult, OP.mult)
        nc.sync.dma_start(z_d[:], zt[:])
    return nc


def _get_nc() -> bass.Bass:
    if "nc" not in _CACHE:
        _pin_act_tables()
        nc = _build_program_v2() if VARIANT == 2 else _build_program()
        # Bacc lowering (register allocation, wait legalization) must run
        # before the PJRT path serializes the module.
        nc.finalize()
        _CACHE["nc"] = nc
    return _CACHE["nc"]


def _make_in_maps(u: np.ndarray) -> list[dict]:
    u_pad = np.zeros(T + W, np.float32)
    u_pad[W:] = u
    ush_pad = np.zeros(T + W, np.float32)
    ush_pad[1:] = u_pad[:-1]
    shiftm = np.eye(P, k=1, dtype=np.float32)
    in_maps = []
    if VARIANT == 2:
        # window j of core c covers [c*TC + j*TC2 - W, c*TC + (j+1)*TC2)
        u_pad2 = np.zeros(T + W, np.float32)
        u_pad2[W:] = u
        for c in range(NCORES):
            uws, ushs = [], []
            for j in range(NW):
                lo = c * TC + j * TC2
                uws.append(u_pad2[lo : lo + CW2].reshape(P, C2))
                ushs.append(ush_pad[lo : lo + CW2].reshape(P, C2))
            in_maps.append(
                {
                    "u_win": np.ascontiguousarray(np.concatenate(uws, axis=1)),
                    "u_sh": np.ascontiguousarray(np.concatenate(ushs, axis=1)),
                }
            )
        return in_maps
    for c in range(NCORES):
        lo = c * TC
        in_maps.append(
            {
                "u_win": np.ascontiguousarray(u_pad[lo : lo + CW].reshape(P, C)),
                "u_sh": np.ascontiguousarray(ush_pad[lo : lo + CW].reshape(P, C)),
                "shiftm": shiftm,
            }
        )
    return in_maps


def _assemble(results: list[dict]) -> np.ndarray:
    z = np.zeros(T + 1, np.float32)
    if VARIANT == 2:
        for c in range(NCORES):
            zc = results[c]["z"]
            for j in range(NW):
                lo = c * TC + j * TC2
                zj = zc[:, j * C2 : (j + 1) * C2].reshape(-1)[W:]
                z[lo + 1 : lo + TC2 + 1] = zj
        return z
    for c in range(NCORES):
        z[c * TC + 1 : (c + 1) * TC + 1] = results[c]["z"].reshape(-1)[W:]
    return z


def kernel(u: np.ndarray, _trace: bool = False):
    u = np.asarray(u, dtype=np.float32).reshape(-1)
    assert u.shape[0] == T, u.shape
    in_maps = _make_in_maps(u)
    res = run_bass_kernel_spmd(
        _get_nc(), in_maps, list(range(NCORES)), trace=_trace
    )
    _CACHE["last_result"] = res
    return _assemble(res.results)


# revision 17
# speedup vs baseline: 1.0665x; 1.0091x over previous
"""Trainium2 Bass kernel for nn_OneDimEquivalent (sequential scalar recurrence).

Math: for t = 0..T-1 with state (k, v) starting at (0, 0):
    delta2 = SIG_M^2 k^2 + SIG_I^2 u_t^2
    gi     = G(delta2)          (Gauss-Legendre-64 integral in the reference)
    k'     = (1-r) k + r*SIG_MN*gi*k + r*SIG_NI*gi*v
    v'     = (1-r) v + r u_t
    z_t    = SIG_MW * gi * k'
Output: [0, z_0 .. z_{T-1}]  (length T+1).

Device algorithm (per core, data-parallel over 8 cores):
  The recurrence contracts (|dk'/dk| <= ~0.87), so each core independently
  processes its T/8 slice plus a W-step warmup prefix. Within a core the
  window is laid out as 128 rows x C cols; the nonlinear recurrence is solved
  by Picard iteration: freeze gi along the current trajectory, then the k
  update is a linear first-order recurrence solved in one tensor_tensor_scan
  per iteration (row chaining via previous iteration's row tails, shifted
  across partitions with a tiny PE matmul). gi = G(delta2) is evaluated as
  G0*exp(-F(y)), y = ln(1 + 3*delta2), F a degree-7 polynomial fit of the
  reference's own quadrature (rel err 1.5e-5). 5 iterations converge to the
  fit floor. v is exact: two chained scans.
"""

import os
import sys

import numpy as np

for _p in ("/opt/trn_rl_repo",):
    if _p not in sys.path and os.path.isdir(_p):
        sys.path.insert(0, _p)

import concourse.bass as bass  # noqa: E402
from concourse import bacc  # noqa: E402


def _pin_act_tables() -> None:
    """All three ACT functions used here (Square, Ln, Exp) live together in
    the natural_log_exp_and_others set, but the table-load inserter picks the
    first set containing each function, which alternates sets and reloads the
    ACT table RAMs (~1.3us) twice per iteration. Strip our functions from
    every other set (ids keep their positions) so one load serves the whole
    kernel."""
    if getattr(bacc, "_act_tables_pinned", False):
        return
    from concourse.hw_specs import get_activation_tables as _orig

    AF = mybir.ActivationFunctionType
    mine = {AF.Square, AF.Ln, AF.Exp}

    def pinned(arch):
        tabs = _orig(arch)
        out = {}
        for name, fns in tabs.items():
            if name == "natural_log_exp_and_others":
                out[name] = fns
            else:
                out[name] = fns - mine
        return out

    bacc.get_activation_tables = pinned
    bacc._act_tables_pinned = True
import concourse.tile as tile  # noqa: E402
from concourse import mybir  # noqa: E402
from concourse.bass_utils import run_bass_kernel_spmd  # noqa: E402

F32 = mybir.dt.float32

T = 524288
NCORES = 8
TC = T // NCORES          # 65536 outputs per core
P = 128                   # partitions
W = 256                   # warmup prefix (contraction 0.87^256 ~ 0)
CW = TC + W               # window elements per core
C = CW // P               # columns per row (514)
N_ITER = 5

R = 0.2                   # DT / TAU
ONE_MR = 1.0 - R
SIG_M = 1.2               # Square scale for 1.44 k^2
SIG_I = 0.8               # Square scale for 0.64 u^2
BETA = 0.5 / 0.9          # SIG_NI / SIG_MN
C6 = 0.7 / (R * 0.9)      # SIG_MW / (r SIG_MN)
ALPHA = 3.0               # y = ln(1 + ALPHA * delta2)

# g1 = r*SIG_MN*G(delta2) = exp(-F~ + EXP_BIAS), F~ = y*Q(y)
EXP_BIAS = -2.633724671491273
# Horner constants, top-first: h = QH[0]*y; h = (h + QH[i])*y ...
QH = (
    3.907613360649551e-05,
    -0.0005464510002932087,
    0.003347948402461663,
    -0.011995295768374504,
    0.024299061938897772,
    0.0028116659750275206,
    0.33319012156717015,
)

_CACHE: dict = {}

VARIANT = int(os.environ.get("KERNEL_VARIANT", "2"))
NW = 2                    # windows per core (v2)
TC2 = TC // NW            # 32768
CW2 = TC2 + W             # 33024
C2 = CW2 // P             # 258
N_ITER2 = 4


def _build_program_v2() -> bass.Bass:
    """Two interleaved half-windows per core: pipelines the serial chain
    across DVE/ACT/Pool/PE; d2-add and b-mult offloaded to gpsimd."""
    from contextlib import ExitStack

    AF = mybir.ActivationFunctionType
    OP = mybir.AluOpType

    nc = bacc.Bacc()
    u_win_d = nc.declare_dram_parameter("u_win", [P, NW * C2], F32, isOutput=False)
    u_sh_d = nc.declare_dram_parameter("u_sh", [P, NW * C2], F32, isOutput=False)
    z_d = nc.declare_dram_parameter("z", [P, NW * C2], F32, isOutput=True)

    with ExitStack() as ctx:
        tc = ctx.enter_context(tile.TileContext(nc))
        pool = ctx.enter_context(tc.tile_pool(name="main", bufs=1))

        def t(tag, cols=C2):
            return pool.tile([P, cols], F32, tag=tag, name=tag)

        WS = range(NW)
        uw = [t(f"uw{w}") for w in WS]
        ush = [t(f"ush{w}") for w in WS]
        U2 = [t(f"U2_{w}") for w in WS]
        bv = [t(f"bv{w}") for w in WS]
        av = t("av")                      # shared constant 0.8 tile
        vsh = [t(f"vsh{w}") for w in WS]
        V2 = [t(f"V2_{w}") for w in WS]
        k = [t(f"k{w}") for w in WS]
        s = [t(f"s{w}") for w in WS]
        d2 = [t(f"d2_{w}") for w in WS]
        y = [t(f"y{w}") for w in WS]
        ha = [t(f"ha{w}") for w in WS]
        hb = [t(f"hb{w}") for w in WS]
        g1 = [t(f"g1_{w}") for w in WS]
        a = [t(f"a{w}") for w in WS]
        b = [t(f"b{w}") for w in WS]
        zt = [t(f"zt{w}") for w in WS]
        ebias = pool.tile([P, 1], F32, tag="ebias", name="ebias")
        # partition-shifted row tails (row 0 stays 0 = cold window start)
        vtail = [
            pool.tile([P, 1], F32, tag=f"vtail{w}", name=f"vtail{w}") for w in WS
        ]
        ktail = [
            pool.tile([P, 1], F32, tag=f"ktail{w}", name=f"ktail{w}") for w in WS
        ]

        for w in WS:
            cs = slice(w * C2, (w + 1) * C2)
            nc.sync.dma_start(uw[w][:], u_win_d[:, cs])
            nc.sync.dma_start(ush[w][:], u_sh_d[:, cs])
        nc.gpsimd.memset(ebias[:], EXP_BIAS)
        nc.gpsimd.memset(av[:], ONE_MR)

        # ---- setup per window ----
        for w in WS:
            nc.scalar.activation(U2[w][:], uw[w][:], AF.Square, scale=SIG_I)
            nc.vector.tensor_scalar_mul(bv[w][:], ush[w][:], R)
            nc.gpsimd.memset(vtail[w][:], 0.0)
            nc.gpsimd.memset(ktail[w][:], 0.0)
            nc.gpsimd.memset(k[w][:], 0.0)
        # vsh: pass 1 (cold row inits), tail shift, then re-run only the
        # first PASS2 columns with correct inits (0.8^PASS2 ~ 1e-13 beyond).
        PASS2 = 128
        for w in WS:
            nc.vector.tensor_tensor_scan(
                vsh[w][:], av[:], bv[w][:], 0.0, OP.mult, OP.add
            )
        for w in WS:
            nc.sync.dma_start(
                vtail[w][1:P, 0:1], vsh[w][0 : P - 1, C2 - 1 : C2]
            )
        for w in WS:
            nc.vector.tensor_tensor_scan(
                vsh[w][:, 0:PASS2],
                av[:, 0:PASS2],
                bv[w][:, 0:PASS2],
                vtail[w][:],
                OP.mult,
                OP.add,
            )
            nc.vector.tensor_scalar_mul(V2[w][:], vsh[w][:], BETA)

        # ---- Picard iterations: anti-phase software pipeline ----
        # front(w,i): ACT/pool stages; back(w,i): DVE burst. Window 1's
        # front is emitted during window 0's back (and vice versa) so each
        # engine's in-order queue always has ready work.
        def front(w, i):
            if i > 0:
                nc.sync.dma_start(
                    ktail[w][1:P, 0:1], k[w][0 : P - 1, C2 - 1 : C2]
                )
            nc.scalar.activation(s[w][:, 0:1], ktail[w][:], AF.Square, scale=SIG_M)
            nc.scalar.activation(
                s[w][:, 1:C2], k[w][:, 0 : C2 - 1], AF.Square, scale=SIG_M
            )
            nc.gpsimd.tensor_tensor(d2[w][:], s[w][:], U2[w][:], OP.add)
            nc.scalar.activation(y[w][:], d2[w][:], AF.Ln, scale=ALPHA, bias=1.0)

        def back(w, i):
            nc.vector.tensor_scalar_mul(ha[w][:], y[w][:], QH[0])
            src, dst = ha[w], hb[w]
            for q in QH[1:]:
                nc.vector.scalar_tensor_tensor(
                    dst[:], src[:], float(q), y[w][:], OP.add, OP.mult
                )
                src, dst = dst, src
            nc.scalar.activation(g1[w][:], src[:], AF.Exp, scale=-1.0, bias=ebias[:])
            nc.vector.tensor_scalar_add(a[w][:], g1[w][:], ONE_MR)
            nc.gpsimd.tensor_tensor(b[w][:], g1[w][:], V2[w][:], OP.mult)
            nc.vector.tensor_tensor_scan(
                k[w][:], a[w][:], b[w][:], ktail[w][:], OP.mult, OP.add
            )

        front(0, 0)
        for it in range(N_ITER2):
            front(1, it)
            back(0, it)
            if it + 1 < N_ITER2:
                front(0, it + 1)
            back(1, it)

        # ---- output ----
        for w in WS:
            nc.vector.scalar_tensor_tensor(
                zt[w][:], g1[w][:], C6, k[w][:], OP.mult, OP.mult
            )
            nc.sync.dma_start(z_d[:, w * C2 : (w + 1) * C2], zt[w][:])
    return nc


def _build_program() -> bass.Bass:
    from contextlib import ExitStack

    AF = mybir.ActivationFunctionType
    OP = mybir.AluOpType

    nc = bacc.Bacc()
    u_win_d = nc.declare_dram_parameter("u_win", [P, C], F32, isOutput=False)
    u_sh_d = nc.declare_dram_parameter("u_sh", [P, C], F32, isOutput=False)
    sm_d = nc.declare_dram_parameter("shiftm", [P, P], F32, isOutput=False)
    z_d = nc.declare_dram_parameter("z", [P, C], F32, isOutput=True)

    with ExitStack() as ctx:
        tc = ctx.enter_context(tile.TileContext(nc))
        pool = ctx.enter_context(tc.tile_pool(name="main", bufs=1))
        psum = ctx.enter_context(tc.tile_pool(name="ps", bufs=2, space="PSUM"))

        def big(tag):
            return pool.tile([P, C], F32, tag=tag, name=tag)

        uw = big("uw")
        ush = big("ush")
        U2 = big("U2")
        bv = big("bv")
        av = big("av")
        vsh = big("vsh")
        V2 = big("V2")
        k = big("k")
        s = big("s")
        d2 = big("d2")
        y = big("y")
        ha = big("ha")
        hb = big("hb")
        g1 = big("g1")
        a = big("a")
        b = big("b")
        zt = big("zt")
        sm = pool.tile([P, P], F32, tag="sm", name="sm")
        ebias = pool.tile([P, 1], F32, tag="ebias", name="ebias")

        nc.sync.dma_start(uw[:], u_win_d[:])
        nc.sync.dma_start(ush[:], u_sh_d[:])
        nc.sync.dma_start(sm[:], sm_d[:])

        # ---- setup: U2 = 0.64 u^2 ; vsh = v_{t-1}; V2 = BETA*vsh ----
        nc.vector.memset(ebias[:], EXP_BIAS)
        nc.scalar.activation(U2[:], uw[:], AF.Square, scale=SIG_I)
        nc.vector.tensor_scalar_mul(bv[:], ush[:], R)
        nc.vector.memset(av[:], ONE_MR)
        nc.vector.tensor_tensor_scan(vsh[:], av[:], bv[:], 0.0, OP.mult, OP.add)
        pv = psum.tile([P, 1], F32, tag="pv", name="pv")
        nc.tensor.matmul(pv[:], sm[:], vsh[:, C - 1 : C], start=True, stop=True)
        nc.vector.tensor_tensor_scan(vsh[:], av[:], bv[:], pv[:], OP.mult, OP.add)
        nc.vector.tensor_scalar_mul(V2[:], vsh[:], BETA)
        nc.vector.memset(k[:], 0.0)

        # ---- Picard iterations ----
        for _it in range(N_ITER):
            pt = psum.tile([P, 1], F32, tag="pt", name="pt")
            nc.tensor.matmul(pt[:], sm[:], k[:, C - 1 : C], start=True, stop=True)
            nc.scalar.activation(s[:, 0:1], pt[:], AF.Square, scale=SIG_M)
            nc.scalar.activation(s[:, 1:C], k[:, 0 : C - 1], AF.Square, scale=SIG_M)
            nc.vector.tensor_tensor(d2[:], s[:], U2[:], OP.add)
            nc.scalar.activation(y[:], d2[:], AF.Ln, scale=ALPHA, bias=1.0)
            nc.vector.tensor_scalar_mul(ha[:], y[:], QH[0])
            src, dst = ha, hb
            for q in QH[1:]:
                nc.vector.scalar_tensor_tensor(
                    dst[:], src[:], float(q), y[:], OP.add, OP.mult
                )
                src, dst = dst, src
            nc.scalar.activation(g1[:], src[:], AF.Exp, scale=-1.0, bias=ebias[:])
            nc.vector.tensor_scalar_add(a[:], g1[:], ONE_MR)
            nc.vector.tensor_tensor(b[:], g1[:], V2[:], OP.mult)
            nc.vector.tensor_tensor_scan(k[:], a[:], b[:], pt[:], OP.mult, OP.add)

        # ---- output: z = (g1 * C6) * k ----
        nc.vector.scalar_tensor_tensor(zt[:], g1[:], C6, k[:], OP.mult, OP.mult)
        nc.sync.dma_start(z_d[:], zt[:])
    return nc


def _get_nc() -> bass.Bass:
    if "nc" not in _CACHE:
        _pin_act_tables()
        nc = _build_program_v2() if VARIANT == 2 else _build_program()
        # Bacc lowering (register allocation, wait legalization) must run
        # before the PJRT path serializes the module.
        nc.finalize()
        _CACHE["nc"] = nc
    return _CACHE["nc"]


def _make_in_maps(u: np.ndarray) -> list[dict]:
    u_pad = np.zeros(T + W, np.float32)
    u_pad[W:] = u
    ush_pad = np.zeros(T + W, np.float32)
    ush_pad[1:] = u_pad[:-1]
    shiftm = np.eye(P, k=1, dtype=np.float32)
    in_maps = []
    if VARIANT == 2:
        # window j of core c covers [c*TC + j*TC2 - W, c*TC + (j+1)*TC2)
        u_pad2 = np.zeros(T + W, np.float32)
        u_pad2[W:] = u
        for c in range(NCORES):
            uws, ushs = [], []
            for j in range(NW):
                lo = c * TC + j * TC2
                uws.append(u_pad2[lo : lo + CW2].reshape(P, C2))
                ushs.append(ush_pad[lo : lo + CW2].reshape(P, C2))
            in_maps.append(
                {
                    "u_win": np.ascontiguousarray(np.concatenate(uws, axis=1)),
                    "u_sh": np.ascontiguousarray(np.concatenate(ushs, axis=1)),
                }
            )
        return in_maps
    for c in range(NCORES):
        lo = c * TC
        in_maps.append(
            {
                "u_win": np.ascontiguousarray(u_pad[lo : lo + CW].reshape(P, C)),
                "u_sh": np.ascontiguousarray(ush_pad[lo : lo + CW].reshape(P, C)),
                "shiftm": shiftm,
            }
        )
    return in_maps


def _assemble(results: list[dict]) -> np.ndarray:
    z = np.zeros(T + 1, np.float32)
    if VARIANT == 2:
        for c in range(NCORES):
            zc = results[c]["z"]
            for j in range(NW):
                lo = c * TC + j * TC2
                zj = zc[:, j * C2 : (j + 1) * C2].reshape(-1)[W:]
                z[lo + 1 : lo + TC2 + 1] = zj
        return z
    for c in range(NCORES):
        z[c * TC + 1 : (c + 1) * TC + 1] = results[c]["z"].reshape(-1)[W:]
    return z


def kernel(u: np.ndarray, _trace: bool = False):
    u = np.asarray(u, dtype=np.float32).reshape(-1)
    assert u.shape[0] == T, u.shape
    in_maps = _make_in_maps(u)
    res = run_bass_kernel_spmd(
        _get_nc(), in_maps, list(range(NCORES)), trace=_trace
    )
    _CACHE["last_result"] = res
    return _assemble(res.results)


# revision 20
# speedup vs baseline: 1.2197x; 1.1437x over previous
"""Trainium2 Bass kernel for nn_OneDimEquivalent (sequential scalar recurrence).

Math: for t = 0..T-1 with state (k, v) starting at (0, 0):
    delta2 = SIG_M^2 k^2 + SIG_I^2 u_t^2
    gi     = G(delta2)          (Gauss-Legendre-64 integral in the reference)
    k'     = (1-r) k + r*SIG_MN*gi*k + r*SIG_NI*gi*v
    v'     = (1-r) v + r u_t
    z_t    = SIG_MW * gi * k'
Output: [0, z_0 .. z_{T-1}]  (length T+1).

Device algorithm (per core, data-parallel over 8 cores):
  The recurrence contracts (|dk'/dk| <= ~0.87), so each core independently
  processes its T/8 slice plus a W-step warmup prefix. Within a core the
  window is laid out as 128 rows x C cols; the nonlinear recurrence is solved
  by Picard iteration: freeze gi along the current trajectory, then the k
  update is a linear first-order recurrence solved in one tensor_tensor_scan
  per iteration (row chaining via previous iteration's row tails, shifted
  across partitions with a tiny PE matmul). gi = G(delta2) is evaluated as
  G0*exp(-F(y)), y = ln(1 + 3*delta2), F a degree-7 polynomial fit of the
  reference's own quadrature (rel err 1.5e-5). 5 iterations converge to the
  fit floor. v is exact: two chained scans.
"""

import os
import sys

import numpy as np

for _p in ("/opt/trn_rl_repo",):
    if _p not in sys.path and os.path.isdir(_p):
        sys.path.insert(0, _p)

import concourse.bass as bass  # noqa: E402
from concourse import bacc  # noqa: E402


def _pin_act_tables() -> None:
    """All three ACT functions used here (Square, Ln, Exp) live together in
    the natural_log_exp_and_others set, but the table-load inserter picks the
    first set containing each function, which alternates sets and reloads the
    ACT table RAMs (~1.3us) twice per iteration. Strip our functions from
    every other set (ids keep their positions) so one load serves the whole
    kernel."""
    if getattr(bacc, "_act_tables_pinned", False):
        return
    from concourse.hw_specs import get_activation_tables as _orig

    AF = mybir.ActivationFunctionType
    mine = {AF.Square, AF.Ln, AF.Exp}

    def pinned(arch):
        tabs = _orig(arch)
        out = {}
        for name, fns in tabs.items():
            if name == "natural_log_exp_and_others":
                out[name] = fns
            else:
                out[name] = fns - mine
        return out

    bacc.get_activation_tables = pinned
    bacc._act_tables_pinned = True
import concourse.tile as tile  # noqa: E402
from concourse import mybir  # noqa: E402
from concourse.bass_utils import run_bass_kernel_spmd  # noqa: E402

F32 = mybir.dt.float32

T = 524288
NCORES = 8
TC = T // NCORES          # 65536 outputs per core
P = 128                   # partitions
W = 256                   # warmup prefix (contraction 0.87^256 ~ 0)
CW = TC + W               # window elements per core
C = CW // P               # columns per row (514)
N_ITER = 5

R = 0.2                   # DT / TAU
ONE_MR = 1.0 - R
SIG_M = 1.2               # Square scale for 1.44 k^2
SIG_I = 0.8               # Square scale for 0.64 u^2
BETA = 0.5 / 0.9          # SIG_NI / SIG_MN
C6 = 0.7 / (R * 0.9)      # SIG_MW / (r SIG_MN)
ALPHA = 3.0               # y = ln(1 + ALPHA * delta2)

# g1 = r*SIG_MN*G(delta2) = exp(-F~ + EXP_BIAS), F~ = y*Q(y)
EXP_BIAS = -2.633724671491273
# Horner constants, top-first: h = QH[0]*y; h = (h + QH[i])*y ...
QH = (
    3.907613360649551e-05,
    -0.0005464510002932087,
    0.003347948402461663,
    -0.011995295768374504,
    0.024299061938897772,
    0.0028116659750275206,
    0.33319012156717015,
)

_CACHE: dict = {}

VARIANT = int(os.environ.get("KERNEL_VARIANT", "2"))
NW = 2                    # windows per core (v2)
TC2 = TC // NW            # 32768
CW2 = TC2 + W             # 33024
C2 = CW2 // P             # 258
N_ITER2 = 4


def _build_program_v2() -> bass.Bass:
    """Two interleaved half-windows per core: pipelines the serial chain
    across DVE/ACT/Pool/PE; d2-add and b-mult offloaded to gpsimd."""
    from contextlib import ExitStack

    AF = mybir.ActivationFunctionType
    OP = mybir.AluOpType

    nc = bacc.Bacc()
    u_win_d = nc.declare_dram_parameter("u_win", [P, NW * C2], F32, isOutput=False)
    u_sh_d = nc.declare_dram_parameter("u_sh", [P, NW * C2], F32, isOutput=False)
    z_d = nc.declare_dram_parameter("z", [P, NW * C2], F32, isOutput=True)

    with ExitStack() as ctx:
        tc = ctx.enter_context(tile.TileContext(nc))
        pool = ctx.enter_context(tc.tile_pool(name="main", bufs=1))

        def t(tag, cols=C2):
            return pool.tile([P, cols], F32, tag=tag, name=tag)

        WS = range(NW)
        uw = [t(f"uw{w}") for w in WS]
        ush = [t(f"ush{w}") for w in WS]
        U2 = [t(f"U2_{w}") for w in WS]
        bv = [t(f"bv{w}") for w in WS]
        av = t("av")                      # shared constant 0.8 tile
        vsh = [t(f"vsh{w}") for w in WS]
        V2 = [t(f"V2_{w}") for w in WS]
        k = [t(f"k{w}") for w in WS]
        s = [t(f"s{w}") for w in WS]
        d2 = [t(f"d2_{w}") for w in WS]
        y = [t(f"y{w}") for w in WS]
        ha = [t(f"ha{w}") for w in WS]
        hb = [t(f"hb{w}") for w in WS]
        g1 = [t(f"g1_{w}") for w in WS]
        a = [t(f"a{w}") for w in WS]
        b = [t(f"b{w}") for w in WS]
        zt = [t(f"zt{w}") for w in WS]
        ebias = pool.tile([P, 1], F32, tag="ebias", name="ebias")
        obias = pool.tile([P, 1], F32, tag="obias", name="obias")
        # partition-shifted row tails (row 0 stays 0 = cold window start).
        # ktail is double-buffered: the scan init uses the fresh tail (exact
        # row chaining) while the g-eval's column 0 reads the previous
        # iteration's tail, so the tail DMA never gates the ACT/pool front.
        vtail = [
            pool.tile([P, 1], F32, tag=f"vtail{w}", name=f"vtail{w}") for w in WS
        ]
        ktail = [
            [
                pool.tile([P, 1], F32, tag=f"ktail{w}_{j}", name=f"ktail{w}_{j}")
                for j in range(2)
            ]
            for w in WS
        ]

        for w in WS:
            cs = slice(w * C2, (w + 1) * C2)
            nc.sync.dma_start(uw[w][:], u_win_d[:, cs])
            nc.sync.dma_start(ush[w][:], u_sh_d[:, cs])
        nc.gpsimd.memset(ebias[:], EXP_BIAS)
        nc.gpsimd.memset(obias[:], ONE_MR)
        nc.gpsimd.memset(av[:], ONE_MR)

        # ---- setup per window ----
        for w in WS:
            nc.scalar.activation(U2[w][:], uw[w][:], AF.Square, scale=SIG_I)
            nc.vector.tensor_scalar_mul(bv[w][:], ush[w][:], R)
            nc.gpsimd.memset(vtail[w][:], 0.0)
            nc.gpsimd.memset(ktail[w][0][:], 0.0)
            nc.gpsimd.memset(ktail[w][1][:], 0.0)
            nc.gpsimd.memset(k[w][:], 0.0)
        # vsh: pass 1 (cold row inits), tail shift, then re-run only the
        # first PASS2 columns with correct inits (0.8^PASS2 ~ 1e-13 beyond).
        PASS2 = 128
        for w in WS:
            nc.vector.tensor_tensor_scan(
                vsh[w][:], av[:], bv[w][:], 0.0, OP.mult, OP.add
            )
        for w in WS:
            nc.sync.dma_start(
                vtail[w][1:P, 0:1], vsh[w][0 : P - 1, C2 - 1 : C2]
            )
        for w in WS:
            nc.vector.tensor_tensor_scan(
                vsh[w][:, 0:PASS2],
                av[:, 0:PASS2],
                bv[w][:, 0:PASS2],
                vtail[w][:],
                OP.mult,
                OP.add,
            )
            nc.vector.tensor_scalar_mul(V2[w][:], vsh[w][:], BETA)

        # ---- Picard iterations: anti-phase software pipeline ----
        # front(w,i): ACT/pool stages; back(w,i): DVE burst. Window 1's
        # front is emitted during window 0's back (and vice versa) so each
        # engine's in-order queue always has ready work.
        def front(w, i):
            fresh, stale = ktail[w][i % 2], ktail[w][(i + 1) % 2]
            if i > 0:
                nc.sync.dma_start(fresh[1:P, 0:1], k[w][0 : P - 1, C2 - 1 : C2])
            nc.scalar.activation(s[w][:, 0:1], stale[:], AF.Square, scale=SIG_M)
            nc.scalar.activation(
                s[w][:, 1:C2], k[w][:, 0 : C2 - 1], AF.Square, scale=SIG_M
            )
            nc.gpsimd.tensor_tensor(d2[w][:], s[w][:], U2[w][:], OP.add)
            nc.scalar.activation(y[w][:], d2[w][:], AF.Ln, scale=ALPHA, bias=1.0)

        def back(w, i):
            fresh = ktail[w][i % 2]
            nc.scalar.mul(ha[w][:], y[w][:], float(QH[0]))
            src, dst = ha[w], hb[w]
            for q in QH[1:]:
                nc.vector.scalar_tensor_tensor(
                    dst[:], src[:], float(q), y[w][:], OP.add, OP.mult
                )
                src, dst = dst, src
            nc.scalar.activation(g1[w][:], src[:], AF.Exp, scale=-1.0, bias=ebias[:])
            nc.scalar.activation(
                a[w][:], g1[w][:], AF.Identity, bias=obias[:], scale=1.0
            )
            nc.gpsimd.tensor_tensor(b[w][:], g1[w][:], V2[w][:], OP.mult)
            nc.vector.tensor_tensor_scan(
                k[w][:], a[w][:], b[w][:], fresh[:], OP.mult, OP.add
            )

        front(0, 0)
        for it in range(N_ITER2):
            front(1, it)
            back(0, it)
            if it + 1 < N_ITER2:
                front(0, it + 1)
            back(1, it)

        # ---- output ----
        for w in WS:
            nc.vector.scalar_tensor_tensor(
                zt[w][:], g1[w][:], C6, k[w][:], OP.mult, OP.mult
            )
            nc.sync.dma_start(z_d[:, w * C2 : (w + 1) * C2], zt[w][:])
    return nc


def _build_program() -> bass.Bass:
    from contextlib import ExitStack

    AF = mybir.ActivationFunctionType
    OP = mybir.AluOpType

    nc = bacc.Bacc()
    u_win_d = nc.declare_dram_parameter("u_win", [P, C], F32, isOutput=False)
    u_sh_d = nc.declare_dram_parameter("u_sh", [P, C], F32, isOutput=False)
    sm_d = nc.declare_dram_parameter("shiftm", [P, P], F32, isOutput=False)
    z_d = nc.declare_dram_parameter("z", [P, C], F32, isOutput=True)

    with ExitStack() as ctx:
        tc = ctx.enter_context(tile.TileContext(nc))
        pool = ctx.enter_context(tc.tile_pool(name="main", bufs=1))
        psum = ctx.enter_context(tc.tile_pool(name="ps", bufs=2, space="PSUM"))

        def big(tag):
            return pool.tile([P, C], F32, tag=tag, name=tag)

        uw = big("uw")
        ush = big("ush")
        U2 = big("U2")
        bv = big("bv")
        av = big("av")
        vsh = big("vsh")
        V2 = big("V2")
        k = big("k")
        s = big("s")
        d2 = big("d2")
        y = big("y")
        ha = big("ha")
        hb = big("hb")
        g1 = big("g1")
        a = big("a")
        b = big("b")
        zt = big("zt")
        sm = pool.tile([P, P], F32, tag="sm", name="sm")
        ebias = pool.tile([P, 1], F32, tag="ebias", name="ebias")

        nc.sync.dma_start(uw[:], u_win_d[:])
        nc.sync.dma_start(ush[:], u_sh_d[:])
        nc.sync.dma_start(sm[:], sm_d[:])

        # ---- setup: U2 = 0.64 u^2 ; vsh = v_{t-1}; V2 = BETA*vsh ----
        nc.vector.memset(ebias[:], EXP_BIAS)
        nc.scalar.activation(U2[:], uw[:], AF.Square, scale=SIG_I)
        nc.vector.tensor_scalar_mul(bv[:], ush[:], R)
        nc.vector.memset(av[:], ONE_MR)
        nc.vector.tensor_tensor_scan(vsh[:], av[:], bv[:], 0.0, OP.mult, OP.add)
        pv = psum.tile([P, 1], F32, tag="pv", name="pv")
        nc.tensor.matmul(pv[:], sm[:], vsh[:, C - 1 : C], start=True, stop=True)
        nc.vector.tensor_tensor_scan(vsh[:], av[:], bv[:], pv[:], OP.mult, OP.add)
        nc.vector.tensor_scalar_mul(V2[:], vsh[:], BETA)
        nc.vector.memset(k[:], 0.0)

        # ---- Picard iterations ----
        for _it in range(N_ITER):
            pt = psum.tile([P, 1], F32, tag="pt", name="pt")
            nc.tensor.matmul(pt[:], sm[:], k[:, C - 1 : C], start=True, stop=True)
            nc.scalar.activation(s[:, 0:1], pt[:], AF.Square, scale=SIG_M)
            nc.scalar.activation(s[:, 1:C], k[:, 0 : C - 1], AF.Square, scale=SIG_M)
            nc.vector.tensor_tensor(d2[:], s[:], U2[:], OP.add)
            nc.scalar.activation(y[:], d2[:], AF.Ln, scale=ALPHA, bias=1.0)
            nc.vector.tensor_scalar_mul(ha[:], y[:], QH[0])
            src, dst = ha, hb
            for q in QH[1:]:
                nc.vector.scalar_tensor_tensor(
                    dst[:], src[:], float(q), y[:], OP.add, OP.mult
                )
                src, dst = dst, src
            nc.scalar.activation(g1[:], src[:], AF.Exp, scale=-1.0, bias=ebias[:])
            nc.vector.tensor_scalar_add(a[:], g1[:], ONE_MR)
            nc.vector.tensor_tensor(b[:], g1[:], V2[:], OP.mult)
            nc.vector.tensor_tensor_scan(k[:], a[:], b[:], pt[:], OP.mult, OP.add)

        # ---- output: z = (g1 * C6) * k ----
        nc.vector.scalar_tensor_tensor(zt[:], g1[:], C6, k[:], OP.mult, OP.mult)
        nc.sync.dma_start(z_d[:], zt[:])
    return nc


def _get_nc() -> bass.Bass:
    if "nc" not in _CACHE:
        _pin_act_tables()
        nc = _build_program_v2() if VARIANT == 2 else _build_program()
        # Bacc lowering (register allocation, wait legalization) must run
        # before the PJRT path serializes the module.
        nc.finalize()
        _CACHE["nc"] = nc
    return _CACHE["nc"]


def _make_in_maps(u: np.ndarray) -> list[dict]:
    u_pad = np.zeros(T + W, np.float32)
    u_pad[W:] = u
    ush_pad = np.zeros(T + W, np.float32)
    ush_pad[1:] = u_pad[:-1]
    shiftm = np.eye(P, k=1, dtype=np.float32)
    in_maps = []
    if VARIANT == 2:
        # window j of core c covers [c*TC + j*TC2 - W, c*TC + (j+1)*TC2)
        u_pad2 = np.zeros(T + W, np.float32)
        u_pad2[W:] = u
        for c in range(NCORES):
            uws, ushs = [], []
            for j in range(NW):
                lo = c * TC + j * TC2
                uws.append(u_pad2[lo : lo + CW2].reshape(P, C2))
                ushs.append(ush_pad[lo : lo + CW2].reshape(P, C2))
            in_maps.append(
                {
                    "u_win": np.ascontiguousarray(np.concatenate(uws, axis=1)),
                    "u_sh": np.ascontiguousarray(np.concatenate(ushs, axis=1)),
                }
            )
        return in_maps
    for c in range(NCORES):
        lo = c * TC
        in_maps.append(
            {
                "u_win": np.ascontiguousarray(u_pad[lo : lo + CW].reshape(P, C)),
                "u_sh": np.ascontiguousarray(ush_pad[lo : lo + CW].reshape(P, C)),
                "shiftm": shiftm,
            }
        )
    return in_maps


def _assemble(results: list[dict]) -> np.ndarray:
    z = np.zeros(T + 1, np.float32)
    if VARIANT == 2:
        for c in range(NCORES):
            zc = results[c]["z"]
            for j in range(NW):
                lo = c * TC + j * TC2
                zj = zc[:, j * C2 : (j + 1) * C2].reshape(-1)[W:]
                z[lo + 1 : lo + TC2 + 1] = zj
        return z
    for c in range(NCORES):
        z[c * TC + 1 : (c + 1) * TC + 1] = results[c]["z"].reshape(-1)[W:]
    return z


def kernel(u: np.ndarray, _trace: bool = False):
    u = np.asarray(u, dtype=np.float32).reshape(-1)
    assert u.shape[0] == T, u.shape
    in_maps = _make_in_maps(u)
    res = run_bass_kernel_spmd(
        _get_nc(), in_maps, list(range(NCORES)), trace=_trace
    )
    _CACHE["last_result"] = res
    return _assemble(res.results)


# revision 25
# speedup vs baseline: 1.3269x; 1.0879x over previous
"""Trainium2 Bass kernel for nn_OneDimEquivalent (sequential scalar recurrence).

Math: for t = 0..T-1 with state (k, v) starting at (0, 0):
    delta2 = SIG_M^2 k^2 + SIG_I^2 u_t^2
    gi     = G(delta2)          (Gauss-Legendre-64 integral in the reference)
    k'     = (1-r) k + r*SIG_MN*gi*k + r*SIG_NI*gi*v
    v'     = (1-r) v + r u_t
    z_t    = SIG_MW * gi * k'
Output: [0, z_0 .. z_{T-1}]  (length T+1).

Device algorithm (per core, data-parallel over 8 cores):
  The recurrence contracts (|dk'/dk| <= ~0.87), so each core independently
  processes its T/8 slice plus a W-step warmup prefix. Within a core the
  window is laid out as 128 rows x C cols; the nonlinear recurrence is solved
  by Picard iteration: freeze gi along the current trajectory, then the k
  update is a linear first-order recurrence solved in one tensor_tensor_scan
  per iteration (row chaining via previous iteration's row tails, shifted
  across partitions with a tiny PE matmul). gi = G(delta2) is evaluated as
  G0*exp(-F(y)), y = ln(1 + 3*delta2), F a degree-7 polynomial fit of the
  reference's own quadrature (rel err 1.5e-5). 5 iterations converge to the
  fit floor. v is exact: two chained scans.
"""

import os
import sys

import numpy as np

for _p in ("/opt/trn_rl_repo",):
    if _p not in sys.path and os.path.isdir(_p):
        sys.path.insert(0, _p)

import concourse.bass as bass  # noqa: E402
from concourse import bacc  # noqa: E402


def _pin_act_tables() -> None:
    """All three ACT functions used here (Square, Ln, Exp) live together in
    the natural_log_exp_and_others set, but the table-load inserter picks the
    first set containing each function, which alternates sets and reloads the
    ACT table RAMs (~1.3us) twice per iteration. Strip our functions from
    every other set (ids keep their positions) so one load serves the whole
    kernel."""
    if getattr(bacc, "_act_tables_pinned", False):
        return
    from concourse.hw_specs import get_activation_tables as _orig

    AF = mybir.ActivationFunctionType
    mine = {AF.Square, AF.Ln, AF.Exp}

    def pinned(arch):
        tabs = _orig(arch)
        out = {}
        for name, fns in tabs.items():
            if name == "natural_log_exp_and_others":
                out[name] = fns
            else:
                out[name] = fns - mine
        return out

    bacc.get_activation_tables = pinned
    bacc._act_tables_pinned = True
import concourse.tile as tile  # noqa: E402
from concourse import mybir  # noqa: E402
from concourse.bass_utils import run_bass_kernel_spmd  # noqa: E402

F32 = mybir.dt.float32

T = 524288
NCORES = 8
TC = T // NCORES          # 65536 outputs per core
P = 128                   # partitions
W = 256                   # warmup prefix (contraction 0.87^256 ~ 0)
CW = TC + W               # window elements per core
C = CW // P               # columns per row (514)
N_ITER = 5

R = 0.2                   # DT / TAU
ONE_MR = 1.0 - R
SIG_M = 1.2               # Square scale for 1.44 k^2
SIG_I = 0.8               # Square scale for 0.64 u^2
BETA = 0.5 / 0.9          # SIG_NI / SIG_MN
C6 = 0.7 / (R * 0.9)      # SIG_MW / (r SIG_MN)
ALPHA = 3.0               # y = ln(1 + ALPHA * delta2)

# g1 = r*SIG_MN*G(delta2) = exp(-F~ + EXP_BIAS), F~ = y*Q(y)
# Two fits of the same quadrature: degree 7 (G rel err 1.5e-5) and degree 5
# (1.2e-4). Horner constants top-first: h = QH[0]*y; h = (h + QH[i])*y ...
_FITS = {
    7: (
        -2.633724671491273,
        (
            3.907613360649551e-05,
            -0.0005464510002932087,
            0.003347948402461663,
            -0.011995295768374504,
            0.024299061938897772,
            0.0028116659750275206,
            0.33319012156717015,
        ),
    ),
    5: (
        -2.6336410694433043,
        (
            3.84568478e-04,
            -4.20625544e-03,
            1.42558513e-02,
            8.32087145e-03,
            3.32500260e-01,
        ),
    ),
}
J_DEG = int(os.environ.get("KERNEL_J", "7"))
EXP_BIAS, QH = _FITS[J_DEG]

_CACHE: dict = {}

VARIANT = int(os.environ.get("KERNEL_VARIANT", "2"))
NW = 2                    # windows per core (v2)
TC2 = TC // NW            # 32768
CW2 = TC2 + W             # 33024
C2 = CW2 // P             # 258
N_ITER2 = 4


def _build_program_v2() -> bass.Bass:
    """Two interleaved half-windows per core: pipelines the serial chain
    across DVE/ACT/Pool/PE; d2-add and b-mult offloaded to gpsimd."""
    from contextlib import ExitStack

    AF = mybir.ActivationFunctionType
    OP = mybir.AluOpType

    nc = bacc.Bacc()
    u_win_d = nc.declare_dram_parameter("u_win", [P, NW * C2], F32, isOutput=False)
    u_sh_d = nc.declare_dram_parameter("u_sh", [P, NW * C2], F32, isOutput=False)
    z_d = nc.declare_dram_parameter("z", [P, NW * C2], F32, isOutput=True)

    with ExitStack() as ctx:
        tc = ctx.enter_context(tile.TileContext(nc))
        pool = ctx.enter_context(tc.tile_pool(name="main", bufs=1))

        def t(tag, cols=C2):
            return pool.tile([P, cols], F32, tag=tag, name=tag)

        WS = range(NW)
        uw = [t(f"uw{w}") for w in WS]
        ush = [t(f"ush{w}") for w in WS]
        U2 = [t(f"U2_{w}") for w in WS]
        bv = [t(f"bv{w}") for w in WS]
        av = t("av")                      # shared constant 0.8 tile
        vsh = [t(f"vsh{w}") for w in WS]
        V2 = [t(f"V2_{w}") for w in WS]
        k = [t(f"k{w}") for w in WS]
        s = [t(f"s{w}") for w in WS]
        d2 = [t(f"d2_{w}") for w in WS]
        y = [t(f"y{w}") for w in WS]
        ha = [t(f"ha{w}") for w in WS]
        hb = [t(f"hb{w}") for w in WS]
        g1 = [t(f"g1_{w}") for w in WS]
        a = [t(f"a{w}") for w in WS]
        b = [t(f"b{w}") for w in WS]
        zt = [t(f"zt{w}") for w in WS]
        ebias = pool.tile([P, 1], F32, tag="ebias", name="ebias")
        obias = pool.tile([P, 1], F32, tag="obias", name="obias")
        # partition-shifted row tails (row 0 stays 0 = cold window start).
        # ktail is double-buffered: the scan init uses the fresh tail (exact
        # row chaining) while the g-eval's column 0 reads the previous
        # iteration's tail, so the tail DMA never gates the ACT/pool front.
        vtail = [
            pool.tile([P, 1], F32, tag=f"vtail{w}", name=f"vtail{w}") for w in WS
        ]
        ktail = [
            [
                pool.tile([P, 1], F32, tag=f"ktail{w}_{j}", name=f"ktail{w}_{j}")
                for j in range(2)
            ]
            for w in WS
        ]

        for w in WS:
            cs = slice(w * C2, (w + 1) * C2)
            nc.sync.dma_start(uw[w][:], u_win_d[:, cs])
            nc.sync.dma_start(ush[w][:], u_sh_d[:, cs])
        nc.gpsimd.memset(ebias[:], EXP_BIAS)
        nc.gpsimd.memset(obias[:], ONE_MR)
        nc.gpsimd.memset(av[:], ONE_MR)

        # ---- setup per window ----
        for w in WS:
            nc.scalar.activation(U2[w][:], uw[w][:], AF.Square, scale=SIG_I)
            nc.scalar.mul(bv[w][:], ush[w][:], R)
            nc.gpsimd.memset(vtail[w][:], 0.0)
            nc.gpsimd.memset(ktail[w][0][:], 0.0)
            nc.gpsimd.memset(ktail[w][1][:], 0.0)
        # vsh: pass 1 (cold row inits), tail shift, then re-run only the
        # first PASS2 columns with correct inits (0.8^PASS2 ~ 1e-13 beyond).
        PASS2 = 128
        for w in WS:
            nc.vector.tensor_tensor_scan(
                vsh[w][:], av[:], bv[w][:], 0.0, OP.mult, OP.add
            )
        for w in WS:
            nc.sync.dma_start(
                vtail[w][1:P, 0:1], vsh[w][0 : P - 1, C2 - 1 : C2]
            )
        for w in WS:
            nc.vector.tensor_tensor_scan(
                vsh[w][:, 0:PASS2],
                av[:, 0:PASS2],
                bv[w][:, 0:PASS2],
                vtail[w][:],
                OP.mult,
                OP.add,
            )
            nc.scalar.mul(V2[w][:], vsh[w][:], BETA)

        # ---- Picard iterations: anti-phase software pipeline ----
        # front(w,i): ACT/pool stages; back(w,i): DVE burst. Window 1's
        # front is emitted during window 0's back (and vice versa) so each
        # engine's in-order queue always has ready work.
        def front(w, i):
            if i == 0:
                # k == 0, so delta2 == U2 directly.
                nc.scalar.activation(y[w][:], U2[w][:], AF.Ln, scale=ALPHA, bias=1.0)
                return
            fresh, stale = ktail[w][i % 2], ktail[w][(i + 1) % 2]
            nc.sync.dma_start(fresh[1:P, 0:1], k[w][0 : P - 1, C2 - 1 : C2])
            nc.scalar.activation(s[w][:, 0:1], stale[:], AF.Square, scale=SIG_M)
            nc.scalar.activation(
                s[w][:, 1:C2], k[w][:, 0 : C2 - 1], AF.Square, scale=SIG_M
            )
            nc.gpsimd.tensor_tensor(d2[w][:], s[w][:], U2[w][:], OP.add)
            nc.scalar.activation(y[w][:], d2[w][:], AF.Ln, scale=ALPHA, bias=1.0)

        def back(w, i):
            fresh = ktail[w][i % 2]
            nc.scalar.mul(ha[w][:], y[w][:], float(QH[0]))
            src, dst = ha[w], hb[w]
            for q in QH[1:]:
                nc.vector.scalar_tensor_tensor(
                    dst[:], src[:], float(q), y[w][:], OP.add, OP.mult
                )
                src, dst = dst, src
            nc.scalar.activation(g1[w][:], src[:], AF.Exp, scale=-1.0, bias=ebias[:])
            nc.scalar.activation(
                a[w][:], g1[w][:], AF.Identity, bias=obias[:], scale=1.0
            )
            nc.gpsimd.tensor_tensor(b[w][:], g1[w][:], V2[w][:], OP.mult)
            nc.vector.tensor_tensor_scan(
                k[w][:], a[w][:], b[w][:], fresh[:], OP.mult, OP.add
            )

        def emit_z(w):
            nc.vector.scalar_tensor_tensor(
                zt[w][:], g1[w][:], C6, k[w][:], OP.mult, OP.mult
            )
            nc.sync.dma_start(z_d[:, w * C2 : (w + 1) * C2], zt[w][:])

        front(0, 0)
        for it in range(N_ITER2):
            front(1, it)
            back(0, it)
            if it + 1 < N_ITER2:
                front(0, it + 1)
            else:
                emit_z(0)
            back(1, it)
        emit_z(1)
    return nc


def _build_program() -> bass.Bass:
    from contextlib import ExitStack

    AF = mybir.ActivationFunctionType
    OP = mybir.AluOpType

    nc = bacc.Bacc()
    u_win_d = nc.declare_dram_parameter("u_win", [P, C], F32, isOutput=False)
    u_sh_d = nc.declare_dram_parameter("u_sh", [P, C], F32, isOutput=False)
    sm_d = nc.declare_dram_parameter("shiftm", [P, P], F32, isOutput=False)
    z_d = nc.declare_dram_parameter("z", [P, C], F32, isOutput=True)

    with ExitStack() as ctx:
        tc = ctx.enter_context(tile.TileContext(nc))
        pool = ctx.enter_context(tc.tile_pool(name="main", bufs=1))
        psum = ctx.enter_context(tc.tile_pool(name="ps", bufs=2, space="PSUM"))

        def big(tag):
            return pool.tile([P, C], F32, tag=tag, name=tag)

        uw = big("uw")
        ush = big("ush")
        U2 = big("U2")
        bv = big("bv")
        av = big("av")
        vsh = big("vsh")
        V2 = big("V2")
        k = big("k")
        s = big("s")
        d2 = big("d2")
        y = big("y")
        ha = big("ha")
        hb = big("hb")
        g1 = big("g1")
        a = big("a")
        b = big("b")
        zt = big("zt")
        sm = pool.tile([P, P], F32, tag="sm", name="sm")
        ebias = pool.tile([P, 1], F32, tag="ebias", name="ebias")

        nc.sync.dma_start(uw[:], u_win_d[:])
        nc.sync.dma_start(ush[:], u_sh_d[:])
        nc.sync.dma_start(sm[:], sm_d[:])

        # ---- setup: U2 = 0.64 u^2 ; vsh = v_{t-1}; V2 = BETA*vsh ----
        nc.vector.memset(ebias[:], EXP_BIAS)
        nc.scalar.activation(U2[:], uw[:], AF.Square, scale=SIG_I)
        nc.vector.tensor_scalar_mul(bv[:], ush[:], R)
        nc.vector.memset(av[:], ONE_MR)
        nc.vector.tensor_tensor_scan(vsh[:], av[:], bv[:], 0.0, OP.mult, OP.add)
        pv = psum.tile([P, 1], F32, tag="pv", name="pv")
        nc.tensor.matmul(pv[:], sm[:], vsh[:, C - 1 : C], start=True, stop=True)
        nc.vector.tensor_tensor_scan(vsh[:], av[:], bv[:], pv[:], OP.mult, OP.add)
        nc.vector.tensor_scalar_mul(V2[:], vsh[:], BETA)
        nc.vector.memset(k[:], 0.0)

        # ---- Picard iterations ----
        for _it in range(N_ITER):
            pt = psum.tile([P, 1], F32, tag="pt", name="pt")
            nc.tensor.matmul(pt[:], sm[:], k[:, C - 1 : C], start=True, stop=True)
            nc.scalar.activation(s[:, 0:1], pt[:], AF.Square, scale=SIG_M)
            nc.scalar.activation(s[:, 1:C], k[:, 0 : C - 1], AF.Square, scale=SIG_M)
            nc.vector.tensor_tensor(d2[:], s[:], U2[:], OP.add)
            nc.scalar.activation(y[:], d2[:], AF.Ln, scale=ALPHA, bias=1.0)
            nc.vector.tensor_scalar_mul(ha[:], y[:], QH[0])
            src, dst = ha, hb
            for q in QH[1:]:
                nc.vector.scalar_tensor_tensor(
                    dst[:], src[:], float(q), y[:], OP.add, OP.mult
                )
                src, dst = dst, src
            nc.scalar.activation(g1[:], src[:], AF.Exp, scale=-1.0, bias=ebias[:])
            nc.vector.tensor_scalar_add(a[:], g1[:], ONE_MR)
            nc.vector.tensor_tensor(b[:], g1[:], V2[:], OP.mult)
            nc.vector.tensor_tensor_scan(k[:], a[:], b[:], pt[:], OP.mult, OP.add)

        # ---- output: z = (g1 * C6) * k ----
        nc.vector.scalar_tensor_tensor(zt[:], g1[:], C6, k[:], OP.mult, OP.mult)
        nc.sync.dma_start(z_d[:], zt[:])
    return nc


def _get_nc() -> bass.Bass:
    if "nc" not in _CACHE:
        _pin_act_tables()
        nc = _build_program_v2() if VARIANT == 2 else _build_program()
        # Bacc lowering (register allocation, wait legalization) must run
        # before the PJRT path serializes the module.
        nc.finalize()
        _CACHE["nc"] = nc
    return _CACHE["nc"]


def _make_in_maps(u: np.ndarray) -> list[dict]:
    u_pad = np.zeros(T + W, np.float32)
    u_pad[W:] = u
    ush_pad = np.zeros(T + W, np.float32)
    ush_pad[1:] = u_pad[:-1]
    shiftm = np.eye(P, k=1, dtype=np.float32)
    in_maps = []
    if VARIANT == 2:
        # window j of core c covers [c*TC + j*TC2 - W, c*TC + (j+1)*TC2)
        u_pad2 = np.zeros(T + W, np.float32)
        u_pad2[W:] = u
        for c in range(NCORES):
            uws, ushs = [], []
            for j in range(NW):
                lo = c * TC + j * TC2
                uws.append(u_pad2[lo : lo + CW2].reshape(P, C2))
                ushs.append(ush_pad[lo : lo + CW2].reshape(P, C2))
            in_maps.append(
                {
                    "u_win": np.ascontiguousarray(np.concatenate(uws, axis=1)),
                    "u_sh": np.ascontiguousarray(np.concatenate(ushs, axis=1)),
                }
            )
        return in_maps
    for c in range(NCORES):
        lo = c * TC
        in_maps.append(
            {
                "u_win": np.ascontiguousarray(u_pad[lo : lo + CW].reshape(P, C)),
                "u_sh": np.ascontiguousarray(ush_pad[lo : lo + CW].reshape(P, C)),
                "shiftm": shiftm,
            }
        )
    return in_maps


def _assemble(results: list[dict]) -> np.ndarray:
    z = np.zeros(T + 1, np.float32)
    if VARIANT == 2:
        for c in range(NCORES):
            zc = results[c]["z"]
            for j in range(NW):
                lo = c * TC + j * TC2
                zj = zc[:, j * C2 : (j + 1) * C2].reshape(-1)[W:]
                z[lo + 1 : lo + TC2 + 1] = zj
        return z
    for c in range(NCORES):
        z[c * TC + 1 : (c + 1) * TC + 1] = results[c]["z"].reshape(-1)[W:]
    return z


def kernel(u: np.ndarray, _trace: bool = False):
    u = np.asarray(u, dtype=np.float32).reshape(-1)
    assert u.shape[0] == T, u.shape
    in_maps = _make_in_maps(u)
    res = run_bass_kernel_spmd(
        _get_nc(), in_maps, list(range(NCORES)), trace=_trace
    )
    _CACHE["last_result"] = res
    return _assemble(res.results)


# revision 26
# speedup vs baseline: 1.3360x; 1.0069x over previous
"""Trainium2 Bass kernel for nn_OneDimEquivalent (sequential scalar recurrence).

Math: for t = 0..T-1 with state (k, v) starting at (0, 0):
    delta2 = SIG_M^2 k^2 + SIG_I^2 u_t^2
    gi     = G(delta2)          (Gauss-Legendre-64 integral in the reference)
    k'     = (1-r) k + r*SIG_MN*gi*k + r*SIG_NI*gi*v
    v'     = (1-r) v + r u_t
    z_t    = SIG_MW * gi * k'
Output: [0, z_0 .. z_{T-1}]  (length T+1).

Device algorithm (per core, data-parallel over 8 cores):
  The recurrence contracts (|dk'/dk| <= ~0.87), so each core independently
  processes its T/8 slice plus a W-step warmup prefix. Within a core the
  window is laid out as 128 rows x C cols; the nonlinear recurrence is solved
  by Picard iteration: freeze gi along the current trajectory, then the k
  update is a linear first-order recurrence solved in one tensor_tensor_scan
  per iteration (row chaining via previous iteration's row tails, shifted
  across partitions with a tiny PE matmul). gi = G(delta2) is evaluated as
  G0*exp(-F(y)), y = ln(1 + 3*delta2), F a degree-7 polynomial fit of the
  reference's own quadrature (rel err 1.5e-5). 5 iterations converge to the
  fit floor. v is exact: two chained scans.
"""

import os
import sys

import numpy as np

for _p in ("/opt/trn_rl_repo",):
    if _p not in sys.path and os.path.isdir(_p):
        sys.path.insert(0, _p)

import concourse.bass as bass  # noqa: E402
from concourse import bacc  # noqa: E402


def _pin_act_tables() -> None:
    """All three ACT functions used here (Square, Ln, Exp) live together in
    the natural_log_exp_and_others set, but the table-load inserter picks the
    first set containing each function, which alternates sets and reloads the
    ACT table RAMs (~1.3us) twice per iteration. Strip our functions from
    every other set (ids keep their positions) so one load serves the whole
    kernel."""
    if getattr(bacc, "_act_tables_pinned", False):
        return
    from concourse.hw_specs import get_activation_tables as _orig

    AF = mybir.ActivationFunctionType
    mine = {AF.Square, AF.Ln, AF.Exp}

    def pinned(arch):
        tabs = _orig(arch)
        out = {}
        for name, fns in tabs.items():
            if name == "natural_log_exp_and_others":
                out[name] = fns
            else:
                out[name] = fns - mine
        return out

    bacc.get_activation_tables = pinned
    bacc._act_tables_pinned = True
import concourse.tile as tile  # noqa: E402
from concourse import mybir  # noqa: E402
from concourse.bass_utils import run_bass_kernel_spmd  # noqa: E402

F32 = mybir.dt.float32

T = 524288
NCORES = 8
TC = T // NCORES          # 65536 outputs per core
P = 128                   # partitions
W = 256                   # warmup prefix (contraction 0.87^256 ~ 0)
CW = TC + W               # window elements per core
C = CW // P               # columns per row (514)
N_ITER = 5

R = 0.2                   # DT / TAU
ONE_MR = 1.0 - R
SIG_M = 1.2               # Square scale for 1.44 k^2
SIG_I = 0.8               # Square scale for 0.64 u^2
BETA = 0.5 / 0.9          # SIG_NI / SIG_MN
C6 = 0.7 / (R * 0.9)      # SIG_MW / (r SIG_MN)
ALPHA = 3.0               # y = ln(1 + ALPHA * delta2)

# g1 = r*SIG_MN*G(delta2) = exp(-F~ + EXP_BIAS), F~ = y*Q(y)
# Two fits of the same quadrature: degree 7 (G rel err 1.5e-5) and degree 5
# (1.2e-4). Horner constants top-first: h = QH[0]*y; h = (h + QH[i])*y ...
_FITS = {
    7: (
        -2.633724671491273,
        (
            3.907613360649551e-05,
            -0.0005464510002932087,
            0.003347948402461663,
            -0.011995295768374504,
            0.024299061938897772,
            0.0028116659750275206,
            0.33319012156717015,
        ),
    ),
    5: (
        -2.6336410694433043,
        (
            3.84568478e-04,
            -4.20625544e-03,
            1.42558513e-02,
            8.32087145e-03,
            3.32500260e-01,
        ),
    ),
}
J_DEG = int(os.environ.get("KERNEL_J", "7"))
EXP_BIAS, QH = _FITS[J_DEG]

_CACHE: dict = {}

VARIANT = int(os.environ.get("KERNEL_VARIANT", "2"))
NW = 2                    # windows per core (v2)
TC2 = TC // NW            # 32768
CW2 = TC2 + W             # 33024
C2 = CW2 // P             # 258
N_ITER2 = 4


def _build_program_v2() -> bass.Bass:
    """Two interleaved half-windows per core: pipelines the serial chain
    across DVE/ACT/Pool/PE; d2-add and b-mult offloaded to gpsimd."""
    from contextlib import ExitStack

    AF = mybir.ActivationFunctionType
    OP = mybir.AluOpType

    nc = bacc.Bacc()
    u_win_d = nc.declare_dram_parameter("u_win", [P, NW * C2], F32, isOutput=False)
    u_sh_d = nc.declare_dram_parameter("u_sh", [P, NW * C2], F32, isOutput=False)
    z_d = nc.declare_dram_parameter("z", [P, NW * C2], F32, isOutput=True)

    with ExitStack() as ctx:
        tc = ctx.enter_context(tile.TileContext(nc))
        pool = ctx.enter_context(tc.tile_pool(name="main", bufs=1))

        def t(tag, cols=C2):
            return pool.tile([P, cols], F32, tag=tag, name=tag)

        WS = range(NW)
        uw = [t(f"uw{w}") for w in WS]
        ush = [t(f"ush{w}") for w in WS]
        U2 = [t(f"U2_{w}") for w in WS]
        bv = [t(f"bv{w}") for w in WS]
        av = t("av")                      # shared constant 0.8 tile
        vsh = [t(f"vsh{w}") for w in WS]
        V2 = [t(f"V2_{w}") for w in WS]
        k = [t(f"k{w}") for w in WS]
        s = [t(f"s{w}") for w in WS]
        d2 = [t(f"d2_{w}") for w in WS]
        y = [t(f"y{w}") for w in WS]
        ha = [t(f"ha{w}") for w in WS]
        hb = [t(f"hb{w}") for w in WS]
        g1 = [t(f"g1_{w}") for w in WS]
        a = [t(f"a{w}") for w in WS]
        b = [t(f"b{w}") for w in WS]
        zt = [t(f"zt{w}") for w in WS]
        ebias = pool.tile([P, 1], F32, tag="ebias", name="ebias")
        obias = pool.tile([P, 1], F32, tag="obias", name="obias")
        # partition-shifted row tails (row 0 stays 0 = cold window start).
        # ktail is double-buffered: the scan init uses the fresh tail (exact
        # row chaining) while the g-eval's column 0 reads the previous
        # iteration's tail, so the tail DMA never gates the ACT/pool front.
        vtail = [
            pool.tile([P, 1], F32, tag=f"vtail{w}", name=f"vtail{w}") for w in WS
        ]
        ktail = [
            [
                pool.tile([P, 1], F32, tag=f"ktail{w}_{j}", name=f"ktail{w}_{j}")
                for j in range(2)
            ]
            for w in WS
        ]

        for w in WS:
            cs = slice(w * C2, (w + 1) * C2)
            nc.sync.dma_start(uw[w][:], u_win_d[:, cs])
            nc.sync.dma_start(ush[w][:], u_sh_d[:, cs])
        nc.gpsimd.memset(ebias[:], EXP_BIAS)
        nc.gpsimd.memset(obias[:], ONE_MR)
        nc.gpsimd.memset(av[:], ONE_MR)

        # ---- setup per window ----
        for w in WS:
            nc.scalar.activation(U2[w][:], uw[w][:], AF.Square, scale=SIG_I)
            nc.vector.tensor_scalar_mul(bv[w][:], ush[w][:], R)
            nc.gpsimd.memset(vtail[w][:], 0.0)
            nc.gpsimd.memset(ktail[w][0][:], 0.0)
            nc.gpsimd.memset(ktail[w][1][:], 0.0)
        # vsh: pass 1 (cold row inits), tail shift, then re-run only the
        # first PASS2 columns with correct inits (0.8^PASS2 ~ 1e-13 beyond).
        PASS2 = 128
        for w in WS:
            nc.vector.tensor_tensor_scan(
                vsh[w][:], av[:], bv[w][:], 0.0, OP.mult, OP.add
            )
        for w in WS:
            nc.sync.dma_start(
                vtail[w][1:P, 0:1], vsh[w][0 : P - 1, C2 - 1 : C2]
            )
        for w in WS:
            nc.vector.tensor_tensor_scan(
                vsh[w][:, 0:PASS2],
                av[:, 0:PASS2],
                bv[w][:, 0:PASS2],
                vtail[w][:],
                OP.mult,
                OP.add,
            )
            nc.vector.tensor_scalar_mul(V2[w][:], vsh[w][:], BETA)

        # ---- Picard iterations: anti-phase software pipeline ----
        # front(w,i): ACT/pool stages; back(w,i): DVE burst. Window 1's
        # front is emitted during window 0's back (and vice versa) so each
        # engine's in-order queue always has ready work.
        def front(w, i):
            if i == 0:
                # k == 0, so delta2 == U2 directly.
                nc.scalar.activation(y[w][:], U2[w][:], AF.Ln, scale=ALPHA, bias=1.0)
                return
            fresh, stale = ktail[w][i % 2], ktail[w][(i + 1) % 2]
            nc.sync.dma_start(fresh[1:P, 0:1], k[w][0 : P - 1, C2 - 1 : C2])
            nc.scalar.activation(s[w][:, 0:1], stale[:], AF.Square, scale=SIG_M)
            nc.scalar.activation(
                s[w][:, 1:C2], k[w][:, 0 : C2 - 1], AF.Square, scale=SIG_M
            )
            nc.gpsimd.tensor_tensor(d2[w][:], s[w][:], U2[w][:], OP.add)
            nc.scalar.activation(y[w][:], d2[w][:], AF.Ln, scale=ALPHA, bias=1.0)

        def back(w, i):
            fresh = ktail[w][i % 2]
            nc.scalar.mul(ha[w][:], y[w][:], float(QH[0]))
            src, dst = ha[w], hb[w]
            for q in QH[1:]:
                nc.vector.scalar_tensor_tensor(
                    dst[:], src[:], float(q), y[w][:], OP.add, OP.mult
                )
                src, dst = dst, src
            nc.scalar.activation(g1[w][:], src[:], AF.Exp, scale=-1.0, bias=ebias[:])
            nc.scalar.activation(
                a[w][:], g1[w][:], AF.Identity, bias=obias[:], scale=1.0
            )
            nc.gpsimd.tensor_tensor(b[w][:], g1[w][:], V2[w][:], OP.mult)
            nc.vector.tensor_tensor_scan(
                k[w][:], a[w][:], b[w][:], fresh[:], OP.mult, OP.add
            )

        def emit_z(w):
            nc.vector.scalar_tensor_tensor(
                zt[w][:], g1[w][:], C6, k[w][:], OP.mult, OP.mult
            )
            nc.sync.dma_start(z_d[:, w * C2 : (w + 1) * C2], zt[w][:])

        front(0, 0)
        for it in range(N_ITER2):
            front(1, it)
            back(0, it)
            if it + 1 < N_ITER2:
                front(0, it + 1)
            else:
                emit_z(0)
            back(1, it)
        emit_z(1)
    return nc


def _build_program() -> bass.Bass:
    from contextlib import ExitStack

    AF = mybir.ActivationFunctionType
    OP = mybir.AluOpType

    nc = bacc.Bacc()
    u_win_d = nc.declare_dram_parameter("u_win", [P, C], F32, isOutput=False)
    u_sh_d = nc.declare_dram_parameter("u_sh", [P, C], F32, isOutput=False)
    sm_d = nc.declare_dram_parameter("shiftm", [P, P], F32, isOutput=False)
    z_d = nc.declare_dram_parameter("z", [P, C], F32, isOutput=True)

    with ExitStack() as ctx:
        tc = ctx.enter_context(tile.TileContext(nc))
        pool = ctx.enter_context(tc.tile_pool(name="main", bufs=1))
        psum = ctx.enter_context(tc.tile_pool(name="ps", bufs=2, space="PSUM"))

        def big(tag):
            return pool.tile([P, C], F32, tag=tag, name=tag)

        uw = big("uw")
        ush = big("ush")
        U2 = big("U2")
        bv = big("bv")
        av = big("av")
        vsh = big("vsh")
        V2 = big("V2")
        k = big("k")
        s = big("s")
        d2 = big("d2")
        y = big("y")
        ha = big("ha")
        hb = big("hb")
        g1 = big("g1")
        a = big("a")
        b = big("b")
        zt = big("zt")
        sm = pool.tile([P, P], F32, tag="sm", name="sm")
        ebias = pool.tile([P, 1], F32, tag="ebias", name="ebias")

        nc.sync.dma_start(uw[:], u_win_d[:])
        nc.sync.dma_start(ush[:], u_sh_d[:])
        nc.sync.dma_start(sm[:], sm_d[:])

        # ---- setup: U2 = 0.64 u^2 ; vsh = v_{t-1}; V2 = BETA*vsh ----
        nc.vector.memset(ebias[:], EXP_BIAS)
        nc.scalar.activation(U2[:], uw[:], AF.Square, scale=SIG_I)
        nc.vector.tensor_scalar_mul(bv[:], ush[:], R)
        nc.vector.memset(av[:], ONE_MR)
        nc.vector.tensor_tensor_scan(vsh[:], av[:], bv[:], 0.0, OP.mult, OP.add)
        pv = psum.tile([P, 1], F32, tag="pv", name="pv")
        nc.tensor.matmul(pv[:], sm[:], vsh[:, C - 1 : C], start=True, stop=True)
        nc.vector.tensor_tensor_scan(vsh[:], av[:], bv[:], pv[:], OP.mult, OP.add)
        nc.vector.tensor_scalar_mul(V2[:], vsh[:], BETA)
        nc.vector.memset(k[:], 0.0)

        # ---- Picard iterations ----
        for _it in range(N_ITER):
            pt = psum.tile([P, 1], F32, tag="pt", name="pt")
            nc.tensor.matmul(pt[:], sm[:], k[:, C - 1 : C], start=True, stop=True)
            nc.scalar.activation(s[:, 0:1], pt[:], AF.Square, scale=SIG_M)
            nc.scalar.activation(s[:, 1:C], k[:, 0 : C - 1], AF.Square, scale=SIG_M)
            nc.vector.tensor_tensor(d2[:], s[:], U2[:], OP.add)
            nc.scalar.activation(y[:], d2[:], AF.Ln, scale=ALPHA, bias=1.0)
            nc.vector.tensor_scalar_mul(ha[:], y[:], QH[0])
            src, dst = ha, hb
            for q in QH[1:]:
                nc.vector.scalar_tensor_tensor(
                    dst[:], src[:], float(q), y[:], OP.add, OP.mult
                )
                src, dst = dst, src
            nc.scalar.activation(g1[:], src[:], AF.Exp, scale=-1.0, bias=ebias[:])
            nc.vector.tensor_scalar_add(a[:], g1[:], ONE_MR)
            nc.vector.tensor_tensor(b[:], g1[:], V2[:], OP.mult)
            nc.vector.tensor_tensor_scan(k[:], a[:], b[:], pt[:], OP.mult, OP.add)

        # ---- output: z = (g1 * C6) * k ----
        nc.vector.scalar_tensor_tensor(zt[:], g1[:], C6, k[:], OP.mult, OP.mult)
        nc.sync.dma_start(z_d[:], zt[:])
    return nc


def _get_nc() -> bass.Bass:
    if "nc" not in _CACHE:
        _pin_act_tables()
        nc = _build_program_v2() if VARIANT == 2 else _build_program()
        # Bacc lowering (register allocation, wait legalization) must run
        # before the PJRT path serializes the module.
        nc.finalize()
        _CACHE["nc"] = nc
    return _CACHE["nc"]


def _make_in_maps(u: np.ndarray) -> list[dict]:
    u_pad = np.zeros(T + W, np.float32)
    u_pad[W:] = u
    ush_pad = np.zeros(T + W, np.float32)
    ush_pad[1:] = u_pad[:-1]
    shiftm = np.eye(P, k=1, dtype=np.float32)
    in_maps = []
    if VARIANT == 2:
        # window j of core c covers [c*TC + j*TC2 - W, c*TC + (j+1)*TC2)
        u_pad2 = np.zeros(T + W, np.float32)
        u_pad2[W:] = u
        for c in range(NCORES):
            uws, ushs = [], []
            for j in range(NW):
                lo = c * TC + j * TC2
                uws.append(u_pad2[lo : lo + CW2].reshape(P, C2))
                ushs.append(ush_pad[lo : lo + CW2].reshape(P, C2))
            in_maps.append(
                {
                    "u_win": np.ascontiguousarray(np.concatenate(uws, axis=1)),
                    "u_sh": np.ascontiguousarray(np.concatenate(ushs, axis=1)),
                }
            )
        return in_maps
    for c in range(NCORES):
        lo = c * TC
        in_maps.append(
            {
                "u_win": np.ascontiguousarray(u_pad[lo : lo + CW].reshape(P, C)),
                "u_sh": np.ascontiguousarray(ush_pad[lo : lo + CW].reshape(P, C)),
                "shiftm": shiftm,
            }
        )
    return in_maps


def _assemble(results: list[dict]) -> np.ndarray:
    z = np.zeros(T + 1, np.float32)
    if VARIANT == 2:
        for c in range(NCORES):
            zc = results[c]["z"]
            for j in range(NW):
                lo = c * TC + j * TC2
                zj = zc[:, j * C2 : (j + 1) * C2].reshape(-1)[W:]
                z[lo + 1 : lo + TC2 + 1] = zj
        return z
    for c in range(NCORES):
        z[c * TC + 1 : (c + 1) * TC + 1] = results[c]["z"].reshape(-1)[W:]
    return z


def kernel(u: np.ndarray, _trace: bool = False):
    u = np.asarray(u, dtype=np.float32).reshape(-1)
    assert u.shape[0] == T, u.shape
    in_maps = _make_in_maps(u)
    res = run_bass_kernel_spmd(
        _get_nc(), in_maps, list(range(NCORES)), trace=_trace
    )
    _CACHE["last_result"] = res
    return _assemble(res.results)
